# revision 1
# baseline (speedup 1.0000x reference)
"""Fused MHA-with-RoPE kernel for one TRN2 chip (8 NeuronCores).

Sharding: core c handles batch b = c//2 and head-group g = c%2 (8 of 16
heads).  Each core:
  phase 1: QKV projections (fp32r matmuls) + RoPE on q/k, q/k/v spilled to
           DRAM in attention-friendly layouts (qT/kT transposed, v natural)
  phase 2: causal attention per head, computed transposed (sT[j,i]) so no
           P transposes are needed; softmax denominator via ones-matmul;
           normalization via PE ones-broadcast + DVE multiply
  phase 3: output projection partial = av @ WoT over this core's 1024 dims,
           per-512-row chunks with pipelined pair-ReduceScatter (issued on
           the Vector queue so gpsimd DMA issue is not blocked)
Host: shards inputs into partition-tiled layouts, reassembles the
RS-interleaved rows.

Self-contained: only numpy + concourse (runtime libs) + the axon boot shim.
"""

import math
import os
import sys
import types
from contextlib import ExitStack

import numpy as np

import concourse.bass as bass
import concourse.tile as tile
from concourse import bacc, mybir
from concourse.bass_utils import run_bass_kernel_spmd

# ---------------------------------------------------------------- constants
B, S, D = 4, 2048, 2048
H, HD = 16, 128
GROUPS = 2            # head groups (cores per batch)
HLOC = H // GROUPS    # heads per core = 8
E = HLOC * HD         # local qkv width = 1024
N_CORES = 8
CORE_IDS = list(range(N_CORES))
SCALE = 1.0 / math.sqrt(HD)
NEG = -1.0e30
ROPE_BASE = 10000.0

F32 = mybir.dt.float32
F32R = mybir.dt.float32r
BF16 = mybir.dt.bfloat16

_cache = {}


def _register_ntff_hook():
    """trn_boot can't register the NTFF profile hook (antenv.axon_hooks is
    missing from this image); recreate it so BASS_TRACE=1 profiling works."""
    if "antenv.axon_hooks" in sys.modules:
        return
    try:
        from trn_agent_boot.trn_boot import _ntff_profile_via_ctypes

        holder = {"h": _ntff_profile_via_ctypes("/opt/axon/libaxon_pjrt.so")}
        mod = types.ModuleType("antenv.axon_hooks")
        mod.get_axon_ntff_profile_hook = lambda: holder["h"]
        mod.set_axon_ntff_profile_hook = lambda h: holder.__setitem__("h", h)
        sys.modules["antenv.axon_hooks"] = mod
    except Exception:
        pass


def _host_tables():
    inv_freq = 1.0 / (ROPE_BASE ** (np.arange(0, HD, 2, dtype=np.float64) / HD))
    pos = np.arange(S, dtype=np.float64)
    freqs = pos[:, None] * inv_freq[None, :]
    emb = np.concatenate([freqs, freqs], axis=-1)        # [S, HD]
    cosT = np.ascontiguousarray(np.cos(emb).T.astype(np.float32))  # [HD, S]
    sinT = np.ascontiguousarray(np.sin(emb).T.astype(np.float32))
    sinF = sinT.copy()
    sinF[: HD // 2] *= -1.0                              # fold rotate_half sign
    return cosT, sinF


def _host_masks():
    # masks[j_local, o, i_local]: 0 if i_local >= o*128 + j_local else NEG
    m = np.empty((128, 4, 512), np.float32)
    jj = np.arange(128)[:, None]
    ii = np.arange(512)[None, :]
    for o in range(4):
        m[:, o, :] = np.where(ii >= o * 128 + jj, 0.0, NEG)
    return m


def _build_nc():
    nc = bacc.Bacc("TRN2", target_bir_lowering=False, debug=False,
                   num_devices=N_CORES)

    # host-pre-tiled inputs: partition-contiguous DMA layouts
    xs_e = nc.dram_tensor("xs", [4, 128, 16, 512], F32R, kind="ExternalInput")
    wq_e = nc.dram_tensor("wq", [HLOC, 128, 16, 128], F32R,
                          kind="ExternalInput")
    wk_e = nc.dram_tensor("wk", [HLOC, 128, 16, 128], F32R,
                          kind="ExternalInput")
    wv_e = nc.dram_tensor("wv", [4, 128, 16, 256], F32R, kind="ExternalInput")
    wo_e = nc.dram_tensor("wo", [128, HLOC, D], F32R, kind="ExternalInput")
    out_e = nc.dram_tensor("out", [4, 512 // GROUPS, D], F32,
                           kind="ExternalOutput")

    cosT_d = nc.inline_tensor(_host_tables()[0], name="cosT")
    sinF_d = nc.inline_tensor(_host_tables()[1], name="sinF")
    masks_d = nc.inline_tensor(_host_masks(), name="masks")
    ones_col_d = nc.inline_tensor(np.ones((128, 1), np.float32), name="ones_col")
    ones_row_d = nc.inline_tensor(np.ones((1, 128), np.float32), name="ones_row")

    with tile.TileContext(nc) as tc, ExitStack() as ctx:
        dram = ctx.enter_context(tc.tile_pool(name="dram", bufs=1, space="DRAM"))
        qh_d = [dram.tile([HD, S], F32R, name=f"qh_d{h}") for h in range(HLOC)]
        kh_d = [dram.tile([HD, S], F32R, name=f"kh_d{h}") for h in range(HLOC)]
        vh_d = [dram.tile([128, 16, HD], F32R, name=f"vh_d{h}")
                for h in range(HLOC)]
        part_d = [dram.tile([512, D], BF16, name=f"part_d{c}")
                  for c in range(4)]
        rs_d = [dram.tile([512 // GROUPS, D], BF16, name=f"rs_d{c}")
                for c in range(4)]

        consts = ctx.enter_context(tc.tile_pool(name="consts", bufs=1))
        masks_sb = consts.tile([128, 4, 512], F32)
        ones_col = consts.tile([128, 1], F32R)
        ones_row = consts.tile([1, 128], F32R)
        nc.gpsimd.dma_start(out=masks_sb[:], in_=masks_d[:])
        nc.gpsimd.dma_start(out=ones_col[:], in_=ones_col_d[:])
        nc.gpsimd.dma_start(out=ones_row[:], in_=ones_row_d[:])

        HF = HD // 2

        # ---------------- phase 1: projections ----------------
        with tc.tile_pool(name="xT", bufs=1) as xT_pool, \
             tc.tile_pool(name="tabs", bufs=1) as tabs:
            cos_sb = tabs.tile([HD, S], F32)
            sinF_sb = tabs.tile([HD, S], F32)
            nc.gpsimd.dma_start(out=cos_sb[:], in_=cosT_d[:])
            nc.gpsimd.dma_start(out=sinF_sb[:], in_=sinF_d[:])

            xs = []
            for sb in range(4):
                xt = xT_pool.tile([128, 16, 512], F32R, name=f"xs{sb}")
                nc.sync.dma_start(out=xt[:], in_=xs_e[sb])
                xs.append(xt)

            ps1_ctx = tc.tile_pool(name="ps1", bufs=4, space="PSUM")
            ps1_all = ps1_ctx.__enter__()
            # q/k projections + RoPE, spilled transposed per head [HD, S]
            with tc.tile_pool(name="wqk", bufs=2) as wqk_pool, \
                 tc.tile_pool(name="rope_wk", bufs=4) as rwk, \
                 tc.tile_pool(name="rot_out", bufs=4) as rout:
                ps1 = ps1_all
                for w_e, o_d, pname in ((wq_e, qh_d, "q"), (wk_e, kh_d, "k")):
                    for m in range(HLOC):
                        w_sb = wqk_pool.tile([128, 16, 128], F32R,
                                             name=f"w{pname}{m}", tag="w")
                        nc.gpsimd.dma_start(out=w_sb[:], in_=w_e[m])
                        for sb in range(4):
                            ps = ps1.tile([128, 512], F32, name="ps_qk",
                                          tag="ps_qk")
                            for dt_ in range(16):
                                nc.tensor.matmul(
                                    ps[:], w_sb[:, dt_, :], xs[sb][:, dt_, :],
                                    start=(dt_ == 0), stop=(dt_ == 15))
                            c_sl = cos_sb[:, bass.ts(sb, 512)]
                            s_sl = sinF_sb[:, bass.ts(sb, 512)]
                            sw = rwk.tile([128, 512], F32, name="sw", tag="sw")
                            nc.scalar.copy(sw[0:HF, :], ps[HF:HD, :])
                            nc.scalar.copy(sw[HF:HD, :], ps[0:HF, :])
                            m1 = rwk.tile([128, 512], F32, name="m1", tag="m1")
                            nc.vector.tensor_mul(m1[:], ps[:], c_sl)
                            m2 = rwk.tile([128, 512], F32, name="m2", tag="m2")
                            nc.vector.tensor_mul(m2[:], sw[:], s_sl)
                            rot = rout.tile([128, 512], F32R, name="rot",
                                            tag="rot")
                            nc.vector.tensor_add(rot[:], m1[:], m2[:])
                            nc.gpsimd.dma_start(
                                out=o_d[m][:, bass.ts(sb, 512)], in_=rot[:])

            # v projection, spilled per head partition-tiled [128, 16, HD]
            with tc.tile_pool(name="wv", bufs=2) as wv_pool, \
                 tc.tile_pool(name="vout", bufs=4) as vout:
                ps1b = ps1_all
                for n in range(4):                     # 256-wide = 2 heads
                    wv_sb = wv_pool.tile([128, 16, 256], F32R, name=f"wv{n}",
                                         tag="wv")
                    nc.sync.dma_start(out=wv_sb[:], in_=wv_e[n])
                    for st in range(16):
                        ps = ps1b.tile([128, 256], F32, name="ps_v", tag="ps_v")
                        for dt_ in range(16):
                            nc.tensor.matmul(
                                ps[:], xs[st // 4][:, dt_,
                                                   bass.ts(st % 4, 128)],
                                wv_sb[:, dt_, :],
                                start=(dt_ == 0), stop=(dt_ == 15))
                        vt = vout.tile([128, 256], F32R, name="vt", tag="vt")
                        nc.scalar.copy(vt[:], ps[:])
                        for half in range(2):
                            nc.gpsimd.dma_start(
                                out=vh_d[2 * n + half][:, st, :],
                                in_=vt[:, bass.ts(half, 128)])

            ps1_ctx.__exit__(None, None, None)

        # ---------------- phase 2: attention ----------------
        avT_pool = ctx.enter_context(tc.tile_pool(name="avT", bufs=1))
        avT_sb = avT_pool.tile([128, HLOC, S], F32R)
        wo_pool = ctx.enter_context(tc.tile_pool(name="wo", bufs=1))
        wo_sb = wo_pool.tile([128, HLOC, D], F32R)
        nc.gpsimd.dma_start(out=wo_sb[:], in_=wo_e[:])

        with tc.tile_pool(name="qh", bufs=2) as qh_pool, \
             tc.tile_pool(name="kh", bufs=2) as kh_pool, \
             tc.tile_pool(name="vh", bufs=2) as vh_pool, \
             tc.tile_pool(name="wk2", bufs=3) as wk2, \
             tc.tile_pool(name="out3", bufs=3) as out3, \
             tc.tile_pool(name="ps2", bufs=2, space="PSUM") as ps2, \
             tc.tile_pool(name="psacc", bufs=2, space="PSUM") as psacc, \
             tc.tile_pool(name="ps3", bufs=2, space="PSUM") as ps3:

            def emit_wo_chunk(cb):
                for i4 in range(4):
                    im = cb * 4 + i4
                    for eb in range(4):
                        ps = ps3.tile([128, 512], F32, name="ps_o",
                                      tag="ps_o")
                        for hh in range(HLOC):
                            nc.tensor.matmul(
                                ps[:], avT_sb[:, hh, bass.ts(im, 128)],
                                wo_sb[:, hh, bass.ts(eb, 512)],
                                start=(hh == 0), stop=(hh == HLOC - 1))
                        po = out3.tile([128, 512], BF16, name="po", tag="po")
                        nc.scalar.copy(po[:], ps[:])
                        nc.sync.dma_start(
                            out=part_d[cb][bass.ts(i4, 128), bass.ts(eb, 512)],
                            in_=po[:])
                nc.gpsimd.collective_compute(
                    "ReduceScatter",
                    mybir.AluOpType.add,
                    replica_groups=[[0, 1], [2, 3], [4, 5], [6, 7]],
                    ins=[part_d[cb][:]],
                    outs=[rs_d[cb][:]],
                )

            for h in range(HLOC):
                q_sb = qh_pool.tile([128, S], F32R, name=f"qh{h}", tag="qh")
                nc.sync.dma_start(out=q_sb[:], in_=qh_d[h][:])
                k_sb = kh_pool.tile([128, S], F32R, name=f"kh{h}", tag="kh")
                nc.sync.dma_start(out=k_sb[:], in_=kh_d[h][:])
                v_sb = vh_pool.tile([128, 16, 128], F32R, name=f"vh{h}",
                                    tag="vh")
                nc.sync.dma_start(out=v_sb[:], in_=vh_d[h][:])
                for ib in range(4):
                    nj = 4 * (ib + 1)
                    den_ps = psacc.tile([1, 512], F32, name="den", tag="den",
                                        bufs=2)
                    av_ps = psacc.tile([128, 512], F32, name="av", tag="av",
                                       bufs=2)
                    for jt in range(nj):
                        s_ps = ps2.tile([128, 512], F32, name="s_ps",
                                        tag="s_ps")
                        nc.tensor.matmul(s_ps[:], k_sb[:, bass.ts(jt, 128)],
                                         q_sb[:, bass.ts(ib, 512)],
                                         start=True, stop=True)
                        o_diag = jt - 4 * ib
                        if o_diag >= 0:
                            msk = wk2.tile([128, 512], F32, name="msk",
                                           tag="msk")
                            nc.vector.tensor_add(msk[:], s_ps[:],
                                                 masks_sb[:, o_diag, :])
                            src = msk
                        else:
                            src = s_ps
                        pT = wk2.tile([128, 512], F32R, name="pT", tag="pT")
                        nc.scalar.activation(
                            pT[:], src[:], mybir.ActivationFunctionType.Exp,
                            scale=SCALE)
                        nc.tensor.matmul(den_ps[:], ones_col[:], pT[:],
                                         start=(jt == 0), stop=(jt == nj - 1))
                        nc.tensor.matmul(av_ps[:], v_sb[:, jt, :], pT[:],
                                         start=(jt == 0), stop=(jt == nj - 1))
                    rden = wk2.tile([1, 512], F32R, name="rden", tag="rden",
                                    bufs=2)
                    with nc.allow_low_precision(reason="f32r rounding only"):
                        nc.vector.reciprocal(rden[:], den_ps[:])
                    bc_sb = wk2.tile([128, 512], F32R, name="bc_sb",
                                     tag="bcs", bufs=2)
                    nc.gpsimd.partition_broadcast(bc_sb[:], rden[:])
                    nc.vector.tensor_mul(avT_sb[:, h, bass.ts(ib, 512)],
                                         av_ps[:], bc_sb[:])

            for cb in range(4):
                emit_wo_chunk(cb)
            for c4 in range(4):
                nc.gpsimd.dma_start(out=out_e[c4], in_=rs_d[c4][:])

    nc.compile()
    return nc


def kernel(x, Wq, Wk, Wv, Wo):
    _register_ntff_hook()
    if "nc" not in _cache:
        _cache["nc"] = _build_nc()
    nc = _cache["nc"]

    in_maps = []
    for c in CORE_IDS:
        b, g = c // GROUPS, c % GROUPS
        sl = slice(g * E, (g + 1) * E)
        xT = np.ascontiguousarray(x[b].T)                       # [D, S]
        in_maps.append({
            "xs": np.ascontiguousarray(
                xT.reshape(16, 128, 4, 512).transpose(2, 1, 0, 3)),
            "wq": np.ascontiguousarray(
                Wq[sl, :].T.reshape(16, 128, HLOC, 128).transpose(2, 1, 0, 3)),
            "wk": np.ascontiguousarray(
                Wk[sl, :].T.reshape(16, 128, HLOC, 128).transpose(2, 1, 0, 3)),
            "wv": np.ascontiguousarray(
                Wv[sl, :].T.reshape(16, 128, 4, 256).transpose(2, 1, 0, 3)),
            "wo": np.ascontiguousarray(
                Wo[:, sl].T.reshape(HLOC, 128, D).transpose(1, 0, 2)),
        })

    trace = bool(os.environ.get("BASS_TRACE"))
    res = run_bass_kernel_spmd(nc, in_maps, CORE_IDS, trace=trace)
    kernel.last_exec_time_ns = res.exec_time_ns
    kernel.last_res = res

    out = np.empty((B, S, D), np.float32)
    half = 512 // GROUPS
    for c in CORE_IDS:
        b, g = c // GROUPS, c % GROUPS
        r = res.results[c]["out"]          # [4, 256, D]
        for ch in range(4):
            lo = ch * 512 + g * half
            out[b, lo:lo + half, :] = r[ch]
    return out


kernel.last_exec_time_ns = None



# revision 8
# speedup vs baseline: 1.1978x; 1.1978x over previous
"""Fused MHA-with-RoPE kernel for one TRN2 chip (8 NeuronCores), v2.

Sharding: core c handles batch b = c//2 and head-group g = c%2 (8 of 16
heads).  All matmul operands are bf16 (fp32 PSUM accumulate), which makes
q/k/v small enough to stay resident in SBUF between phases (no DRAM
spill/reload) and keeps every LDWEIGHTS hidden under its matmul.

  phase 1: QKV projections, sb-outer (x double-buffered per 512-token
           block, w tiles streamed), RoPE fused on the PSUM results,
           q/k stored transposed [hd, S] and v natural [s, hd] in SBUF.
  phase 2: causal attention per (i-block, head), scores computed
           transposed (sT[j,i]) so no PE transposes are needed; softmax
           denominator via ones-matmul accumulated alongside AV; 1/den
           via DVE reciprocal_approx_fast; denominator broadcast via a
           tiny ones-row PE matmul.  Finalize (recip/broadcast/normalize)
           of each head group is deferred past the next group's first
           score tile so the PE never waits on the exp chain.
  phase 3: output projection, interleaved per 512-token chunk inside
           phase 2's i-block loop: chunk cb's 16 Wo tiles are emitted two
           per head during chunk cb+1's attention, then a pair
           ReduceScatter per chunk, so only the last chunk's Wo+RS is on
           the critical-path tail.
Host: shards/transposes/bf16-casts inputs, reassembles RS-interleaved rows.

Self-contained: only numpy/ml_dtypes + concourse + the axon boot shim.
"""

import math
import os
import sys
import types
from contextlib import ExitStack

import ml_dtypes
import numpy as np

import concourse.bass as bass
import concourse.tile as tile
from concourse import bacc, mybir
from concourse.bass_utils import run_bass_kernel_spmd

# ---------------------------------------------------------------- constants
B, S, D = 4, 2048, 2048
H, HD = 16, 128
GROUPS = 2            # head groups (cores per batch)
HLOC = H // GROUPS    # heads per core = 8
N_CORES = 8
CORE_IDS = list(range(N_CORES))
SCALE = 1.0 / math.sqrt(HD)
NEG = -1.0e30
ROPE_BASE = 10000.0

F32 = mybir.dt.float32
BF16 = mybir.dt.bfloat16
BF = ml_dtypes.bfloat16

_cache = {}


def _register_ntff_hook():
    """trn_boot can't register the NTFF profile hook (antenv.axon_hooks is
    missing from this image); recreate it so BASS_TRACE=1 profiling works."""
    if "antenv.axon_hooks" in sys.modules:
        return
    try:
        from trn_agent_boot.trn_boot import _ntff_profile_via_ctypes

        holder = {"h": _ntff_profile_via_ctypes("/opt/axon/libaxon_pjrt.so")}
        mod = types.ModuleType("antenv.axon_hooks")
        mod.get_axon_ntff_profile_hook = lambda: holder["h"]
        mod.set_axon_ntff_profile_hook = lambda h: holder.__setitem__("h", h)
        sys.modules["antenv.axon_hooks"] = mod
    except Exception:
        pass


def _host_tables():
    inv_freq = 1.0 / (ROPE_BASE ** (np.arange(0, HD, 2, dtype=np.float64) / HD))
    pos = np.arange(S, dtype=np.float64)
    freqs = pos[:, None] * inv_freq[None, :]
    emb = np.concatenate([freqs, freqs], axis=-1)        # [S, HD]
    cosT = np.ascontiguousarray(np.cos(emb).T.astype(np.float32))  # [HD, S]
    sinT = np.ascontiguousarray(np.sin(emb).T.astype(np.float32))
    sinF = sinT.copy()
    sinF[: HD // 2] *= -1.0                              # fold rotate_half sign
    return cosT, sinF


def _host_masks():
    # masks[j_local, o, i_local]: 0 if i_local >= o*128 + j_local else NEG
    m = np.empty((128, 4, 512), np.float32)
    jj = np.arange(128)[:, None]
    ii = np.arange(512)[None, :]
    for o in range(4):
        m[:, o, :] = np.where(ii >= o * 128 + jj, 0.0, NEG)
    return m


def _build_nc():
    nc = bacc.Bacc("TRN2", target_bir_lowering=False, debug=False,
                   num_devices=N_CORES)

    # host-pre-tiled bf16 inputs: partition-contiguous DMA layouts
    xs_e = nc.dram_tensor("xs", [4, 128, 16, 512], BF16, kind="ExternalInput")
    wq_e = nc.dram_tensor("wq", [HLOC, 128, 16, 128], BF16,
                          kind="ExternalInput")
    wk_e = nc.dram_tensor("wk", [HLOC, 128, 16, 128], BF16,
                          kind="ExternalInput")
    wv_e = nc.dram_tensor("wv", [128, 16, HLOC * HD], BF16,
                          kind="ExternalInput")
    wo_e = nc.dram_tensor("wo", [128, HLOC, D], BF16, kind="ExternalInput")
    out_e = nc.dram_tensor("out", [4, 512 // GROUPS, D], F32,
                           kind="ExternalOutput")

    cosT_d = nc.inline_tensor(_host_tables()[0], name="cosT")
    sinF_d = nc.inline_tensor(_host_tables()[1], name="sinF")
    masks_d = nc.inline_tensor(_host_masks(), name="masks")

    with tile.TileContext(nc) as tc, ExitStack() as ctx:
        dram = ctx.enter_context(tc.tile_pool(name="dram", bufs=1, space="DRAM"))
        part_d = [dram.tile([512, D], BF16, name=f"part_d{c}")
                  for c in range(4)]
        rs_d = [dram.tile([512 // GROUPS, D], BF16, name=f"rs_d{c}")
                for c in range(4)]

        # persistent across phases: ones vectors + q/k/v in SBUF
        consts = ctx.enter_context(tc.tile_pool(name="consts", bufs=1))
        ones_col = consts.tile([128, 1], BF16)
        nc.vector.memset(ones_col[:], 1.0)

        qkv = ctx.enter_context(tc.tile_pool(name="qkv", bufs=1))
        q_sb = qkv.tile([128, HLOC, S], BF16)     # [hd, h, s]
        k_sb = qkv.tile([128, HLOC, S], BF16)     # [hd, h, s]
        v_sb = qkv.tile([128, 16, HLOC * HD], BF16)  # [s_sub, st, h*hd]

        # ---------------- phase 1: projections ----------------
        with tc.tile_pool(name="tabs", bufs=1) as tabs, \
             tc.tile_pool(name="xT", bufs=2) as xT_pool, \
             tc.tile_pool(name="wqk", bufs=3) as wqk_pool, \
             tc.tile_pool(name="wv", bufs=1) as wv_pool, \
             tc.tile_pool(name="rope", bufs=2) as rope, \
             tc.tile_pool(name="ps1", bufs=4, space="PSUM") as ps1:
            cos_sb = tabs.tile([HD, S], F32)
            sinF_sb = tabs.tile([HD, S], F32)
            nc.gpsimd.dma_start(out=cos_sb[:], in_=cosT_d[:])
            nc.gpsimd.dma_start(out=sinF_sb[:], in_=sinF_d[:])
            wv_sb = wv_pool.tile([128, 16, HLOC * HD], BF16)
            nc.scalar.dma_start(out=wv_sb[:], in_=wv_e[:])

            HF = HD // 2
            for sb in range(4):
                xt = xT_pool.tile([128, 16, 512], BF16, name=f"xt{sb}",
                                  tag="xt")
                for qd in range(4):          # quarter DMAs: early PE start
                    nc.sync.dma_start(out=xt[:, 4 * qd:4 * qd + 4, :],
                                      in_=xs_e[sb, :, 4 * qd:4 * qd + 4, :])
                c_sl = cos_sb[:, bass.ts(sb, 512)]
                s_sl = sinF_sb[:, bass.ts(sb, 512)]
                # q/k projections + RoPE, written transposed [hd, h, s]
                for w_e, o_sb, pname in ((wq_e, q_sb, "q"), (wk_e, k_sb, "k")):
                    for m in range(HLOC):
                        w_t = wqk_pool.tile([128, 16, 128], BF16,
                                            name=f"w{pname}{m}", tag="w")
                        nc.sync.dma_start(out=w_t[:], in_=w_e[m])
                        ps = ps1.tile([128, 512], F32, name="ps_qk",
                                      tag="ps1")
                        for dt_ in range(16):
                            nc.tensor.matmul(
                                ps[:], w_t[:, dt_, :], xt[:, dt_, :],
                                start=(dt_ == 0), stop=(dt_ == 15))
                        sw = rope.tile([128, 512], F32, name="sw", tag="sw")
                        nc.scalar.copy(sw[0:HF, :], ps[HF:HD, :])
                        nc.scalar.copy(sw[HF:HD, :], ps[0:HF, :])
                        m1 = rope.tile([128, 512], F32, name="m1", tag="m1")
                        nc.vector.tensor_mul(m1[:], ps[:], c_sl)
                        m2 = rope.tile([128, 512], F32, name="m2", tag="m2")
                        nc.vector.tensor_mul(m2[:], sw[:], s_sl)
                        nc.vector.tensor_add(
                            o_sb[:, m, bass.ts(sb, 512)], m1[:], m2[:])
                # v projection: x tile stationary, wv moving (512-row MMs)
                for ss in range(4):
                    st = sb * 4 + ss
                    for eh in range(2):
                        ps = ps1.tile([128, 512], F32, name="ps_v", tag="ps1")
                        for dt_ in range(16):
                            nc.tensor.matmul(
                                ps[:], xt[:, dt_, bass.ts(ss, 128)],
                                wv_sb[:, dt_, bass.ts(eh, 512)],
                                start=(dt_ == 0), stop=(dt_ == 15))
                        nc.scalar.copy(v_sb[:, st, bass.ts(eh, 512)], ps[:])

        # ---------------- phase 2+3: attention + Wo + RS ----------------
        with tc.tile_pool(name="wo", bufs=1) as wo_pool, \
             tc.tile_pool(name="avT", bufs=1) as avT_pool, \
             tc.tile_pool(name="msks", bufs=1) as msks_pool, \
             tc.tile_pool(name="p2", bufs=3) as p2, \
             tc.tile_pool(name="p2b", bufs=2) as p2b, \
             tc.tile_pool(name="ps_s", bufs=3, space="PSUM") as ps_s, \
             tc.tile_pool(name="ps_av", bufs=2, space="PSUM") as ps_av, \
             tc.tile_pool(name="ps_den", bufs=1, space="PSUM") as ps_den, \
             tc.tile_pool(name="ps_wo", bufs=2, space="PSUM") as ps_wo:
            wo_sb = wo_pool.tile([128, HLOC, D], BF16)
            nc.scalar.dma_start(out=wo_sb[:], in_=wo_e[:])
            avT_sb = avT_pool.tile([128, HLOC, S], BF16)
            masks_sb = msks_pool.tile([128, 4, 512], F32)
            nc.gpsimd.dma_start(out=masks_sb[:], in_=masks_d[:])

            pending = [None]     # deferred finalize of the previous group

            def flush_pending():
                if pending[0] is not None:
                    pending[0]()
                    pending[0] = None

            def attn_group(h, ib):
                nj = 4 * (ib + 1)
                for jt in range(nj):
                    s_t = ps_s.tile([128, 512], F32, name="s_t", tag="s")
                    nc.tensor.matmul(s_t[:], k_sb[:, h, bass.ts(jt, 128)],
                                     q_sb[:, h, bass.ts(ib, 512)],
                                     start=True, stop=True)
                    o_diag = jt - 4 * ib
                    if o_diag >= 0:
                        msk = p2b.tile([128, 512], F32, name="msk", tag="msk")
                        nc.vector.tensor_add(msk[:], s_t[:],
                                             masks_sb[:, o_diag, :])
                        src = msk
                    else:
                        src = s_t
                    pT = p2.tile([128, 512], BF16, name="pT", tag="pT")
                    nc.scalar.activation(
                        pT[:], src[:], mybir.ActivationFunctionType.Exp,
                        scale=SCALE)
                    if jt == 0:
                        # finalize the previous group between the first exp
                        # and the first den/av matmul: the recip lands before
                        # den's WAR on the (bufs=1) den bank, and the bc
                        # matmul never stalls the PE
                        flush_pending()
                        den_t = ps_den.tile([1, 512], F32, name="den",
                                            tag="den")
                        av_t = ps_av.tile([128, 512], F32, name="av",
                                          tag="av")
                    nc.tensor.matmul(den_t[:], ones_col[:], pT[:],
                                     start=(jt == 0), stop=(jt == nj - 1))
                    nc.tensor.matmul(av_t[:], v_sb[:, jt, bass.ts(h, 128)],
                                     pT[:], start=(jt == 0),
                                     stop=(jt == nj - 1))

                den_f, av_f = den_t, av_t

                def finalize():
                    rden = p2b.tile([1, 512], F32, name="rden", tag="rden")
                    nc.vector.reciprocal_approx_fast(rden[:], den_f[:])
                    bc = p2b.tile([128, 512], F32, name="bc", tag="bc")
                    nc.gpsimd.partition_broadcast(bc[:], rden[:])
                    nc.vector.tensor_mul(avT_sb[:, h, bass.ts(ib, 512)],
                                         av_f[:], bc[:])

                pending[0] = finalize

            def emit_wo_tile(cb, t):
                i4, eb = t // 4, t % 4
                im = cb * 4 + i4
                wps = ps_wo.tile([128, 512], F32, name="wps", tag="wps")
                for hh in range(HLOC):
                    nc.tensor.matmul(
                        wps[:], avT_sb[:, hh, bass.ts(im, 128)],
                        wo_sb[:, hh, bass.ts(eb, 512)],
                        start=(hh == 0), stop=(hh == HLOC - 1))
                po = p2.tile([128, 512], BF16, name="po", tag="po")
                nc.vector.tensor_scalar_mul(po[:], wps[:], 1.0)
                nc.sync.dma_start(
                    out=part_d[cb][bass.ts(i4, 128), bass.ts(eb, 512)],
                    in_=po[:])

            def emit_rs(cb):
                nc.gpsimd.collective_compute(
                    "ReduceScatter",
                    mybir.AluOpType.add,
                    replica_groups=[[0, 1], [2, 3], [4, 5], [6, 7]],
                    ins=[part_d[cb][:]],
                    outs=[rs_d[cb][:]],
                )

            for ib in range(4):
                for h in range(HLOC):
                    attn_group(h, ib)
                    if ib >= 1:           # interleave prev chunk's Wo tiles
                        for t in (2 * h, 2 * h + 1):
                            emit_wo_tile(ib - 1, t)
                        if h == HLOC - 1:
                            emit_rs(ib - 1)
            flush_pending()
            for t in range(16):
                emit_wo_tile(3, t)
            emit_rs(3)
            # final out DMAs at the very end so an RS-completion wait can
            # never block the gpsimd queue mid-kernel
            for cb in range(4):
                nc.gpsimd.dma_start(out=out_e[cb], in_=rs_d[cb][:])

    nc.compile()
    return nc


def kernel(x, Wq, Wk, Wv, Wo):
    _register_ntff_hook()
    if "nc" not in _cache:
        _cache["nc"] = _build_nc()
    nc = _cache["nc"]

    E = HLOC * HD
    in_maps = []
    for c in CORE_IDS:
        b, g = c // GROUPS, c % GROUPS
        sl = slice(g * E, (g + 1) * E)
        xT = np.ascontiguousarray(x[b].T)                       # [D, S]
        in_maps.append({
            "xs": np.ascontiguousarray(
                xT.reshape(16, 128, 4, 512).transpose(2, 1, 0, 3)).astype(BF),
            "wq": np.ascontiguousarray(
                Wq[sl, :].T.reshape(16, 128, HLOC, 128)
                .transpose(2, 1, 0, 3)).astype(BF),
            "wk": np.ascontiguousarray(
                Wk[sl, :].T.reshape(16, 128, HLOC, 128)
                .transpose(2, 1, 0, 3)).astype(BF),
            "wv": np.ascontiguousarray(
                Wv[sl, :].T.reshape(16, 128, E).transpose(1, 0, 2)).astype(BF),
            "wo": np.ascontiguousarray(
                Wo[:, sl].T.reshape(HLOC, 128, D)
                .transpose(1, 0, 2)).astype(BF),
        })

    trace = bool(os.environ.get("BASS_TRACE"))
    res = run_bass_kernel_spmd(nc, in_maps, CORE_IDS, trace=trace)
    kernel.last_exec_time_ns = res.exec_time_ns
    kernel.last_res = res

    out = np.empty((B, S, D), np.float32)
    half = 512 // GROUPS
    for c in CORE_IDS:
        b, g = c // GROUPS, c % GROUPS
        r = res.results[c]["out"]          # [4, 256, D]
        for ch in range(4):
            lo = ch * 512 + g * half
            out[b, lo:lo + half, :] = r[ch]
    return out


kernel.last_exec_time_ns = None


# revision 22
# speedup vs baseline: 1.2085x; 1.0089x over previous
"""Fused MHA-with-RoPE kernel for one TRN2 chip (8 NeuronCores), v2.

Sharding: core c handles batch b = c//2 and head-group g = c%2 (8 of 16
heads).  All matmul operands are bf16 (fp32 PSUM accumulate), which makes
q/k/v small enough to stay resident in SBUF between phases (no DRAM
spill/reload) and keeps every LDWEIGHTS hidden under its matmul.

  phase 1: QKV projections, sb-outer (x double-buffered per 512-token
           block, w tiles streamed), RoPE fused on the PSUM results,
           q/k stored transposed [hd, S] and v natural [s, hd] in SBUF.
  phase 2: causal attention per (i-block, head), scores computed
           transposed (sT[j,i]) so no PE transposes are needed; softmax
           denominator via ones-matmul accumulated alongside AV; 1/den
           via DVE reciprocal_approx_fast; denominator broadcast via a
           tiny ones-row PE matmul.  Finalize (recip/broadcast/normalize)
           of each head group is deferred past the next group's first
           score tile so the PE never waits on the exp chain.
  phase 3: output projection, interleaved per 512-token chunk inside
           phase 2's i-block loop: chunk cb's 16 Wo tiles are emitted two
           per head during chunk cb+1's attention, then a pair
           ReduceScatter per chunk, so only the last chunk's Wo+RS is on
           the critical-path tail.
Host: shards/transposes/bf16-casts inputs, reassembles RS-interleaved rows.

Self-contained: only numpy/ml_dtypes + concourse + the axon boot shim.
"""

import math
import os
import sys
import types
from contextlib import ExitStack

import ml_dtypes
import numpy as np

import concourse.bass as bass
import concourse.tile as tile
from concourse import bacc, mybir
from concourse.bass_utils import run_bass_kernel_spmd

# ---------------------------------------------------------------- constants
B, S, D = 4, 2048, 2048
H, HD = 16, 128
GROUPS = 2            # head groups (cores per batch)
HLOC = H // GROUPS    # heads per core = 8
N_CORES = 8
CORE_IDS = list(range(N_CORES))
SCALE = 1.0 / math.sqrt(HD)
NEG = -1.0e30
ROPE_BASE = 10000.0

F32 = mybir.dt.float32
BF16 = mybir.dt.bfloat16
BF = ml_dtypes.bfloat16

_cache = {}


def _register_ntff_hook():
    """trn_boot can't register the NTFF profile hook (antenv.axon_hooks is
    missing from this image); recreate it so BASS_TRACE=1 profiling works."""
    if "antenv.axon_hooks" in sys.modules:
        return
    try:
        from trn_agent_boot.trn_boot import _ntff_profile_via_ctypes

        holder = {"h": _ntff_profile_via_ctypes("/opt/axon/libaxon_pjrt.so")}
        mod = types.ModuleType("antenv.axon_hooks")
        mod.get_axon_ntff_profile_hook = lambda: holder["h"]
        mod.set_axon_ntff_profile_hook = lambda h: holder.__setitem__("h", h)
        sys.modules["antenv.axon_hooks"] = mod
    except Exception:
        pass


def _host_tables():
    inv_freq = 1.0 / (ROPE_BASE ** (np.arange(0, HD, 2, dtype=np.float64) / HD))
    pos = np.arange(S, dtype=np.float64)
    freqs = pos[:, None] * inv_freq[None, :]
    emb = np.concatenate([freqs, freqs], axis=-1)        # [S, HD]
    cosT = np.ascontiguousarray(np.cos(emb).T.astype(np.float32))  # [HD, S]
    sinT = np.ascontiguousarray(np.sin(emb).T.astype(np.float32))
    sinF = sinT.copy()
    sinF[: HD // 2] *= -1.0                              # fold rotate_half sign
    return cosT, sinF


def _host_masks():
    # masks[j_local, o, i_local]: 0 if i_local >= o*128 + j_local else NEG
    m = np.empty((128, 4, 512), np.float32)
    jj = np.arange(128)[:, None]
    ii = np.arange(512)[None, :]
    for o in range(4):
        m[:, o, :] = np.where(ii >= o * 128 + jj, 0.0, NEG)
    return m


def _build_nc():
    nc = bacc.Bacc("TRN2", target_bir_lowering=False, debug=False,
                   num_devices=N_CORES)

    # host-pre-tiled bf16 inputs: partition-contiguous DMA layouts
    xs_e = nc.dram_tensor("xs", [4, 128, 16, 512], BF16, kind="ExternalInput")
    wq_e = nc.dram_tensor("wq", [HLOC, 128, 16, 128], BF16,
                          kind="ExternalInput")
    wk_e = nc.dram_tensor("wk", [HLOC, 128, 16, 128], BF16,
                          kind="ExternalInput")
    wv_e = nc.dram_tensor("wv", [128, 16, HLOC * HD], BF16,
                          kind="ExternalInput")
    wo_e = nc.dram_tensor("wo", [128, HLOC, D], BF16, kind="ExternalInput")
    # bf16, written directly by the ReduceScatters; host converts to f32.
    # rows: per 512-chunk cb 0-2 -> 256 rows; 256-chunks 3a/3b -> 128 rows
    out_e = nc.dram_tensor("out", [S // GROUPS, D], BF16,
                           kind="ExternalOutput")

    cosT_d = nc.inline_tensor(_host_tables()[0], name="cosT")
    sinF_d = nc.inline_tensor(_host_tables()[1], name="sinF")
    masks_d = nc.inline_tensor(_host_masks(), name="masks")

    with tile.TileContext(nc) as tc, ExitStack() as ctx:
        dram = ctx.enter_context(tc.tile_pool(name="dram", bufs=1, space="DRAM"))
        # chunks 0-2: 512 tokens; 3a/3b: 256 tokens (finer tail)
        CHUNKS = [(0, 512), (512, 512), (1024, 512), (1536, 256), (1792, 256)]
        part_d = [dram.tile([w, D], BF16, name=f"part_d{i}")
                  for i, (_, w) in enumerate(CHUNKS)]
        rs_d = [dram.tile([w // GROUPS, D], BF16, name=f"rs_d{i}")
                for i, (_, w) in enumerate(CHUNKS)]

        # persistent across phases: ones vectors + q/k/v in SBUF
        consts = ctx.enter_context(tc.tile_pool(name="consts", bufs=1))
        ones_col = consts.tile([128, 1], BF16)
        ones_row = consts.tile([1, 128], BF16)
        nc.vector.memset(ones_col[:], 1.0)
        nc.vector.memset(ones_row[:], 1.0)

        qkv = ctx.enter_context(tc.tile_pool(name="qkv", bufs=1))
        q_sb = qkv.tile([128, HLOC, S], BF16)     # [hd, h, s]
        k_sb = qkv.tile([128, HLOC, S], BF16)     # [hd, h, s]
        v_sb = qkv.tile([128, 16, HLOC * HD], BF16)  # [s_sub, st, h*hd]

        # ---------------- phase 1: projections ----------------
        with tc.tile_pool(name="tabs", bufs=1) as tabs, \
             tc.tile_pool(name="xT", bufs=2) as xT_pool, \
             tc.tile_pool(name="wqk", bufs=3) as wqk_pool, \
             tc.tile_pool(name="wv", bufs=1) as wv_pool, \
             tc.tile_pool(name="rope", bufs=2) as rope, \
             tc.tile_pool(name="ps1", bufs=4, space="PSUM") as ps1:
            cos_sb = tabs.tile([HD, S], F32)
            sinF_sb = tabs.tile([HD, S], F32)
            nc.gpsimd.dma_start(out=cos_sb[:], in_=cosT_d[:])
            nc.gpsimd.dma_start(out=sinF_sb[:], in_=sinF_d[:])
            wv_sb = wv_pool.tile([128, 16, HLOC * HD], BF16)
            nc.scalar.dma_start(out=wv_sb[:], in_=wv_e[:])

            HF = HD // 2
            for sb in range(4):
                xt = xT_pool.tile([128, 16, 512], BF16, name=f"xt{sb}",
                                  tag="xt")
                if sb > 0:
                    for qd in range(4):
                        nc.sync.dma_start(out=xt[:, 4 * qd:4 * qd + 4, :],
                                          in_=xs_e[sb, :, 4 * qd:4 * qd + 4, :])
                c_sl = cos_sb[:, bass.ts(sb, 512)]
                s_sl = sinF_sb[:, bass.ts(sb, 512)]
                # q/k projections + RoPE, written transposed [hd, h, s]
                for w_e, o_sb, pname in ((wq_e, q_sb, "q"), (wk_e, k_sb, "k")):
                    for m in range(HLOC):
                        w_t = wqk_pool.tile([128, 16, 128], BF16,
                                            name=f"w{pname}{m}", tag="w")
                        if sb == 0 and pname == "q" and m == 0:
                            # interleave x/w quarter loads so the very first
                            # matmul's deps land in ~3us, not after the full
                            # 2.5MB prefix of the queue
                            for qd in range(4):
                                nc.sync.dma_start(
                                    out=xt[:, 4 * qd:4 * qd + 4, :],
                                    in_=xs_e[0, :, 4 * qd:4 * qd + 4, :])
                                nc.sync.dma_start(
                                    out=w_t[:, 4 * qd:4 * qd + 4, :],
                                    in_=w_e[0, :, 4 * qd:4 * qd + 4, :])
                        else:
                            nc.sync.dma_start(out=w_t[:], in_=w_e[m])
                        ps = ps1.tile([128, 512], F32, name="ps_qk",
                                      tag="ps1")
                        for dt_ in range(16):
                            nc.tensor.matmul(
                                ps[:], w_t[:, dt_, :], xt[:, dt_, :],
                                start=(dt_ == 0), stop=(dt_ == 15))
                        sw = rope.tile([128, 512], F32, name="sw", tag="sw")
                        nc.scalar.copy(sw[0:HF, :], ps[HF:HD, :])
                        nc.scalar.copy(sw[HF:HD, :], ps[0:HF, :])
                        m1 = rope.tile([128, 512], F32, name="m1", tag="m1")
                        nc.vector.tensor_mul(m1[:], ps[:], c_sl)
                        m2 = rope.tile([128, 512], F32, name="m2", tag="m2")
                        nc.vector.tensor_mul(m2[:], sw[:], s_sl)
                        nc.vector.tensor_add(
                            o_sb[:, m, bass.ts(sb, 512)], m1[:], m2[:])
                # v projection: x tile stationary, wv moving (512-row MMs)
                for ss in range(4):
                    st = sb * 4 + ss
                    for eh in range(2):
                        ps = ps1.tile([128, 512], F32, name="ps_v", tag="ps1")
                        for dt_ in range(16):
                            nc.tensor.matmul(
                                ps[:], xt[:, dt_, bass.ts(ss, 128)],
                                wv_sb[:, dt_, bass.ts(eh, 512)],
                                start=(dt_ == 0), stop=(dt_ == 15))
                        nc.scalar.copy(v_sb[:, st, bass.ts(eh, 512)], ps[:])

        # ---------------- phase 2+3: attention + Wo + RS ----------------
        with tc.tile_pool(name="wo", bufs=1) as wo_pool, \
             tc.tile_pool(name="avT", bufs=1) as avT_pool, \
             tc.tile_pool(name="msks", bufs=1) as msks_pool, \
             tc.tile_pool(name="p2", bufs=3) as p2, \
             tc.tile_pool(name="p2b", bufs=2) as p2b, \
             tc.tile_pool(name="ps_s", bufs=3, space="PSUM") as ps_s, \
             tc.tile_pool(name="ps_av", bufs=2, space="PSUM") as ps_av, \
             tc.tile_pool(name="ps_den", bufs=1, space="PSUM") as ps_den, \
             tc.tile_pool(name="ps_wo", bufs=2, space="PSUM") as ps_wo:
            wo_sb = wo_pool.tile([128, HLOC, D], BF16)
            nc.scalar.dma_start(out=wo_sb[:], in_=wo_e[:])
            avT_sb = avT_pool.tile([128, HLOC, S], BF16)
            masks_sb = msks_pool.tile([128, 4, 512], F32)
            nc.gpsimd.dma_start(out=masks_sb[:], in_=masks_d[:])

            # two-stage deferred finalize of the previous group: stage 1
            # (recip + bf16 copy, DVE) flushes at the next group's jt==0;
            # stage 2 (bc matmul + normalize) at jt==1 so the PE reaches the
            # bc matmul well after the DVE chain finished
            pending = [None]
            pending2 = [None]

            def flush_pending():
                if pending[0] is not None:
                    pending[0]()
                    pending[0] = None

            def flush_pending2():
                if pending2[0] is not None:
                    pending2[0]()
                    pending2[0] = None

            def attn_group(h, blk, bw):
                # i-block of width bw tokens starting at token blk
                nj = (blk + bw) // 128
                nd = blk // 128          # first diagonal-band j-tile
                for jt in range(nj):
                    s_t = ps_s.tile([128, 512], F32, name="s_t", tag="s")
                    nc.tensor.matmul(s_t[:, 0:bw],
                                     k_sb[:, h, bass.ts(jt, 128)],
                                     q_sb[:, h, blk:blk + bw],
                                     start=True, stop=True)
                    o_diag = jt - nd
                    if o_diag >= 0:
                        msk = p2b.tile([128, 512], F32, name="msk", tag="msk")
                        nc.vector.tensor_add(msk[:, 0:bw], s_t[:, 0:bw],
                                             masks_sb[:, o_diag, 0:bw])
                        src = msk
                    else:
                        src = s_t
                    pT = p2.tile([128, 512], BF16, name="pT", tag="pT")
                    nc.scalar.activation(
                        pT[:, 0:bw], src[:, 0:bw],
                        mybir.ActivationFunctionType.Exp, scale=SCALE)
                    if jt == 0:
                        # stage-1 finalize of the previous group lands before
                        # den's WAR on the (bufs=1) den bank
                        flush_pending()
                        den_t = ps_den.tile([1, 512], F32, name="den",
                                            tag="den")
                        av_t = ps_av.tile([128, 512], F32, name="av",
                                          tag="av")
                    nc.tensor.matmul(den_t[:, 0:bw], ones_col[:],
                                     pT[:, 0:bw], start=(jt == 0),
                                     stop=(jt == nj - 1))
                    nc.tensor.matmul(av_t[:, 0:bw],
                                     v_sb[:, jt, bass.ts(h, 128)],
                                     pT[:, 0:bw], start=(jt == 0),
                                     stop=(jt == nj - 1))
                    if jt == 1:
                        flush_pending2()

                den_f, av_f = den_t, av_t

                def finalize1():
                    rden = p2b.tile([1, 512], F32, name="rden", tag="rden")
                    nc.vector.reciprocal_approx_fast(rden[:, 0:bw],
                                                     den_f[:, 0:bw])
                    rden_bf = p2b.tile([1, 512], BF16, name="rden_bf",
                                       tag="rden_bf")
                    nc.vector.tensor_scalar_mul(rden_bf[:, 0:bw],
                                                rden[:, 0:bw], 1.0)
                    av_sb = p2b.tile([128, 512], F32, name="av_sb",
                                     tag="av_sb")
                    nc.vector.tensor_scalar_mul(av_sb[:, 0:bw],
                                                av_f[:, 0:bw], 1.0)

                    def finalize2():
                        bc = ps_s.tile([128, 512], F32, name="bc", tag="s")
                        nc.tensor.matmul(bc[:, 0:bw], ones_row[:],
                                         rden_bf[:, 0:bw],
                                         start=True, stop=True)
                        # DVE may read at most one PSUM operand: av_sb is
                        # the SBUF copy, bc stays in PSUM
                        nc.vector.tensor_mul(avT_sb[:, h, blk:blk + bw],
                                             av_sb[:, 0:bw], bc[:, 0:bw])

                    pending2[0] = finalize2

                pending[0] = finalize1

            def emit_wo_tile(cb, t):
                base, w = CHUNKS[cb]
                ic, eb = t // 4, t % 4
                im = base // 128 + ic
                wps = ps_wo.tile([128, 512], F32, name="wps", tag="wps")
                for hh in range(HLOC):
                    nc.tensor.matmul(
                        wps[:], avT_sb[:, hh, bass.ts(im, 128)],
                        wo_sb[:, hh, bass.ts(eb, 512)],
                        start=(hh == 0), stop=(hh == HLOC - 1))
                po = p2.tile([128, 512], BF16, name="po", tag="po")
                nc.vector.tensor_scalar_mul(po[:], wps[:], 1.0)
                nc.sync.dma_start(
                    out=part_d[cb][bass.ts(ic, 128), bass.ts(eb, 512)],
                    in_=po[:])

            def emit_rs(cb):
                nc.gpsimd.collective_compute(
                    "ReduceScatter",
                    mybir.AluOpType.add,
                    replica_groups=[[0, 1], [2, 3], [4, 5], [6, 7]],
                    ins=[part_d[cb][:]],
                    outs=[rs_d[cb][:]],
                )

            # i-blocks: three 512-wide, then two 256-wide (finer tail);
            # chunk cb's Wo tiles+RS are interleaved into block cb+1
            BLOCKS = [(0, 512), (512, 512), (1024, 512), (1536, 256),
                      (1792, 256)]
            for bi, (blk, bw) in enumerate(BLOCKS):
                ntiles = CHUNKS[bi - 1][1] // 32 if bi >= 1 else 0
                for h in range(HLOC):
                    attn_group(h, blk, bw)
                    if bi >= 1:       # interleave prev chunk's Wo tiles
                        per = ntiles // HLOC
                        for t in range(per * h, per * (h + 1)):
                            emit_wo_tile(bi - 1, t)
                        if h == HLOC - 1:
                            emit_rs(bi - 1)
            flush_pending()
            flush_pending2()
            for t in range(8):
                emit_wo_tile(4, t)
            emit_rs(4)
            # out DMAs live on the gpsimd queue, which nothing
            # latency-critical shares anymore: even when the scheduler hoists
            # out(cb) right behind RS(cb), the RS-done wait at the queue head
            # only delays later RS issues, which have >=60us of slack
            for cb in range(5):
                base, w = CHUNKS[cb]
                o0 = base // GROUPS
                nc.gpsimd.dma_start(out=out_e[o0:o0 + w // GROUPS, :],
                                    in_=rs_d[cb][:])

    nc.compile()
    return nc


def kernel(x, Wq, Wk, Wv, Wo):
    _register_ntff_hook()
    if "nc" not in _cache:
        _cache["nc"] = _build_nc()
    nc = _cache["nc"]

    E = HLOC * HD
    in_maps = []
    for c in CORE_IDS:
        b, g = c // GROUPS, c % GROUPS
        sl = slice(g * E, (g + 1) * E)
        xT = np.ascontiguousarray(x[b].T)                       # [D, S]
        in_maps.append({
            "xs": np.ascontiguousarray(
                xT.reshape(16, 128, 4, 512).transpose(2, 1, 0, 3)).astype(BF),
            "wq": np.ascontiguousarray(
                Wq[sl, :].T.reshape(16, 128, HLOC, 128)
                .transpose(2, 1, 0, 3)).astype(BF),
            "wk": np.ascontiguousarray(
                Wk[sl, :].T.reshape(16, 128, HLOC, 128)
                .transpose(2, 1, 0, 3)).astype(BF),
            "wv": np.ascontiguousarray(
                Wv[sl, :].T.reshape(16, 128, E).transpose(1, 0, 2)).astype(BF),
            "wo": np.ascontiguousarray(
                Wo[:, sl].T.reshape(HLOC, 128, D)
                .transpose(1, 0, 2)).astype(BF),
        })

    trace = bool(os.environ.get("BASS_TRACE"))
    res = run_bass_kernel_spmd(nc, in_maps, CORE_IDS, trace=trace)
    kernel.last_exec_time_ns = res.exec_time_ns
    kernel.last_res = res

    out = np.empty((B, S, D), np.float32)
    chunks = [(0, 512), (512, 512), (1024, 512), (1536, 256), (1792, 256)]
    for c in CORE_IDS:
        b, g = c // GROUPS, c % GROUPS
        r = np.asarray(res.results[c]["out"]).astype(np.float32)  # [1024, D]
        for base, w in chunks:
            half = w // GROUPS
            lo = base + g * half
            out[b, lo:lo + half, :] = r[base // GROUPS:base // GROUPS + half]
    return out


kernel.last_exec_time_ns = None


# revision 29
# speedup vs baseline: 1.2176x; 1.0075x over previous
"""Fused MHA-with-RoPE kernel for one TRN2 chip (8 NeuronCores), v2.

Sharding: core c handles batch b = c//2 and head-group g = c%2 (8 of 16
heads).  All matmul operands are bf16 (fp32 PSUM accumulate), which makes
q/k/v small enough to stay resident in SBUF between phases (no DRAM
spill/reload) and keeps every LDWEIGHTS hidden under its matmul.

  phase 1: QKV projections, sb-outer (x double-buffered per 512-token
           block, w tiles streamed), RoPE fused on the PSUM results,
           q/k stored transposed [hd, S] and v natural [s, hd] in SBUF.
  phase 2: causal attention per (i-block, head), scores computed
           transposed (sT[j,i]) so no PE transposes are needed; softmax
           denominator via ones-matmul accumulated alongside AV; 1/den
           via DVE reciprocal_approx_fast; denominator broadcast via a
           tiny ones-row PE matmul.  Finalize (recip/broadcast/normalize)
           of each head group is deferred past the next group's first
           score tile so the PE never waits on the exp chain.
  phase 3: output projection, interleaved per 512-token chunk inside
           phase 2's i-block loop: chunk cb's 16 Wo tiles are emitted two
           per head during chunk cb+1's attention, then a pair
           ReduceScatter per chunk, so only the last chunk's Wo+RS is on
           the critical-path tail.
Host: shards/transposes/bf16-casts inputs, reassembles RS-interleaved rows.

Self-contained: only numpy/ml_dtypes + concourse + the axon boot shim.
"""

import math
import os
import sys
import types
from contextlib import ExitStack

import ml_dtypes
import numpy as np

import concourse.bass as bass
import concourse.tile as tile
from concourse import bacc, mybir
from concourse.bass_utils import run_bass_kernel_spmd

# ---------------------------------------------------------------- constants
B, S, D = 4, 2048, 2048
H, HD = 16, 128
GROUPS = 2            # head groups (cores per batch)
HLOC = H // GROUPS    # heads per core = 8
N_CORES = 8
CORE_IDS = list(range(N_CORES))
SCALE = 1.0 / math.sqrt(HD)
NEG = -1.0e30
ROPE_BASE = 10000.0

F32 = mybir.dt.float32
BF16 = mybir.dt.bfloat16
BF = ml_dtypes.bfloat16

_cache = {}


def _register_ntff_hook():
    """trn_boot can't register the NTFF profile hook (antenv.axon_hooks is
    missing from this image); recreate it so BASS_TRACE=1 profiling works."""
    if "antenv.axon_hooks" in sys.modules:
        return
    try:
        from trn_agent_boot.trn_boot import _ntff_profile_via_ctypes

        holder = {"h": _ntff_profile_via_ctypes("/opt/axon/libaxon_pjrt.so")}
        mod = types.ModuleType("antenv.axon_hooks")
        mod.get_axon_ntff_profile_hook = lambda: holder["h"]
        mod.set_axon_ntff_profile_hook = lambda h: holder.__setitem__("h", h)
        sys.modules["antenv.axon_hooks"] = mod
    except Exception:
        pass


def _host_tables():
    inv_freq = 1.0 / (ROPE_BASE ** (np.arange(0, HD, 2, dtype=np.float64) / HD))
    pos = np.arange(S, dtype=np.float64)
    freqs = pos[:, None] * inv_freq[None, :]
    emb = np.concatenate([freqs, freqs], axis=-1)        # [S, HD]
    cosT = np.ascontiguousarray(np.cos(emb).T.astype(np.float32))  # [HD, S]
    sinT = np.ascontiguousarray(np.sin(emb).T.astype(np.float32))
    sinF = sinT.copy()
    sinF[: HD // 2] *= -1.0                              # fold rotate_half sign
    return cosT, sinF


def _host_masks():
    # masks[j_local, o, i_local]: 0 if i_local >= o*128 + j_local else NEG
    m = np.empty((128, 4, 512), np.float32)
    jj = np.arange(128)[:, None]
    ii = np.arange(512)[None, :]
    for o in range(4):
        m[:, o, :] = np.where(ii >= o * 128 + jj, 0.0, NEG)
    return m


def _build_nc():
    nc = bacc.Bacc("TRN2", target_bir_lowering=False, debug=False,
                   num_devices=N_CORES)

    # host-pre-tiled bf16 inputs: partition-contiguous DMA layouts
    xs_e = nc.dram_tensor("xs", [4, 128, 16, 512], BF16, kind="ExternalInput")
    wq_e = nc.dram_tensor("wq", [HLOC, 128, 16, 128], BF16,
                          kind="ExternalInput")
    wk_e = nc.dram_tensor("wk", [HLOC, 128, 16, 128], BF16,
                          kind="ExternalInput")
    wv_e = nc.dram_tensor("wv", [128, 16, HLOC * HD], BF16,
                          kind="ExternalInput")
    wo_e = nc.dram_tensor("wo", [128, HLOC, D], BF16, kind="ExternalInput")
    # bf16, written directly by the ReduceScatters; host converts to f32.
    # rows: per 512-chunk cb 0-2 -> 256 rows; 256-chunks 3a/3b -> 128 rows
    out_e = nc.dram_tensor("out", [S // GROUPS, D], BF16,
                           kind="ExternalOutput")

    cosT_d = nc.inline_tensor(_host_tables()[0], name="cosT")
    sinF_d = nc.inline_tensor(_host_tables()[1], name="sinF")
    masks_d = nc.inline_tensor(_host_masks(), name="masks")

    with tile.TileContext(nc) as tc, ExitStack() as ctx:
        dram = ctx.enter_context(tc.tile_pool(name="dram", bufs=1, space="DRAM"))
        # chunks 0-2: 512 tokens; 3a/3b: 256 tokens (finer tail)
        CHUNKS = [(0, 512), (512, 512), (1024, 512), (1536, 256), (1792, 256)]
        part_d = [dram.tile([w, D], BF16, name=f"part_d{i}")
                  for i, (_, w) in enumerate(CHUNKS)]
        rs_d = [dram.tile([w // GROUPS, D], BF16, name=f"rs_d{i}")
                for i, (_, w) in enumerate(CHUNKS)]

        # persistent across phases: ones vectors + q/k/v in SBUF
        consts = ctx.enter_context(tc.tile_pool(name="consts", bufs=1))
        ones_col = consts.tile([128, 1], BF16)
        nc.vector.memset(ones_col[:], 1.0)

        qkv = ctx.enter_context(tc.tile_pool(name="qkv", bufs=1))
        q_sb = qkv.tile([128, HLOC, S], BF16)     # [hd, h, s]
        k_sb = qkv.tile([128, HLOC, S], BF16)     # [hd, h, s]
        v_sb = qkv.tile([128, 16, HLOC * HD], BF16)  # [s_sub, st, h*hd]

        # ---------------- phase 1: projections ----------------
        with tc.tile_pool(name="tabs", bufs=1) as tabs, \
             tc.tile_pool(name="xT", bufs=2) as xT_pool, \
             tc.tile_pool(name="wqk", bufs=3) as wqk_pool, \
             tc.tile_pool(name="wv", bufs=1) as wv_pool, \
             tc.tile_pool(name="rope", bufs=2) as rope, \
             tc.tile_pool(name="ps1", bufs=4, space="PSUM") as ps1:
            cos_sb = tabs.tile([HD, S], F32)
            sinF_sb = tabs.tile([HD, S], F32)
            nc.gpsimd.dma_start(out=cos_sb[:], in_=cosT_d[:])
            nc.gpsimd.dma_start(out=sinF_sb[:], in_=sinF_d[:])
            wv_sb = wv_pool.tile([128, 16, HLOC * HD], BF16)
            nc.scalar.dma_start(out=wv_sb[:], in_=wv_e[:])

            HF = HD // 2
            for sb in range(4):
                xt = xT_pool.tile([128, 16, 512], BF16, name=f"xt{sb}",
                                  tag="xt")
                if sb > 0:
                    for qd in range(4):
                        nc.sync.dma_start(out=xt[:, 4 * qd:4 * qd + 4, :],
                                          in_=xs_e[sb, :, 4 * qd:4 * qd + 4, :])
                c_sl = cos_sb[:, bass.ts(sb, 512)]
                s_sl = sinF_sb[:, bass.ts(sb, 512)]
                # q/k projections + RoPE, written transposed [hd, h, s]
                for w_e, o_sb, pname in ((wq_e, q_sb, "q"), (wk_e, k_sb, "k")):
                    for m in range(HLOC):
                        w_t = wqk_pool.tile([128, 16, 128], BF16,
                                            name=f"w{pname}{m}", tag="w")
                        if sb == 0 and pname == "q" and m == 0:
                            # per-dt loads: slice-granular deps let matmul dt
                            # start as soon as its own 160KB arrived
                            for dt_ in range(16):
                                nc.sync.dma_start(
                                    out=xt[:, dt_:dt_ + 1, :],
                                    in_=xs_e[0, :, dt_:dt_ + 1, :])
                                nc.sync.dma_start(
                                    out=w_t[:, dt_:dt_ + 1, :],
                                    in_=w_e[0, :, dt_:dt_ + 1, :])
                        else:
                            nc.sync.dma_start(out=w_t[:], in_=w_e[m])
                        ps = ps1.tile([128, 512], F32, name="ps_qk",
                                      tag="ps1")
                        for dt_ in range(16):
                            nc.tensor.matmul(
                                ps[:], w_t[:, dt_, :], xt[:, dt_, :],
                                start=(dt_ == 0), stop=(dt_ == 15))
                        sw = rope.tile([128, 512], F32, name="sw", tag="sw")
                        nc.scalar.copy(sw[0:HF, :], ps[HF:HD, :])
                        nc.scalar.copy(sw[HF:HD, :], ps[0:HF, :])
                        m1 = rope.tile([128, 512], F32, name="m1", tag="m1")
                        nc.vector.tensor_mul(m1[:], ps[:], c_sl)
                        m2 = rope.tile([128, 512], F32, name="m2", tag="m2")
                        nc.vector.tensor_mul(m2[:], sw[:], s_sl)
                        nc.vector.tensor_add(
                            o_sb[:, m, bass.ts(sb, 512)], m1[:], m2[:])
                # v projection: x tile stationary, wv moving (512-row MMs)
                for ss in range(4):
                    st = sb * 4 + ss
                    for eh in range(2):
                        ps = ps1.tile([128, 512], F32, name="ps_v", tag="ps1")
                        for dt_ in range(16):
                            nc.tensor.matmul(
                                ps[:], xt[:, dt_, bass.ts(ss, 128)],
                                wv_sb[:, dt_, bass.ts(eh, 512)],
                                start=(dt_ == 0), stop=(dt_ == 15))
                        nc.scalar.copy(v_sb[:, st, bass.ts(eh, 512)], ps[:])

        # ---------------- phase 2+3: attention + Wo + RS ----------------
        with tc.tile_pool(name="wo", bufs=1) as wo_pool, \
             tc.tile_pool(name="avT", bufs=1) as avT_pool, \
             tc.tile_pool(name="msks", bufs=1) as msks_pool, \
             tc.tile_pool(name="p2", bufs=3) as p2, \
             tc.tile_pool(name="p2b", bufs=2) as p2b, \
             tc.tile_pool(name="ps_s", bufs=3, space="PSUM") as ps_s, \
             tc.tile_pool(name="ps_av", bufs=2, space="PSUM") as ps_av, \
             tc.tile_pool(name="ps_den", bufs=1, space="PSUM") as ps_den, \
             tc.tile_pool(name="ps_wo", bufs=2, space="PSUM") as ps_wo:
            wo_sb = wo_pool.tile([128, HLOC, D], BF16)
            avT_sb = avT_pool.tile([128, HLOC, S], BF16)
            masks_sb = msks_pool.tile([128, 4, 512], F32)
            # dummy writes depending on phase-1 data gate these bulk loads
            # off the critical first ~30us of input DMA bandwidth (the
            # scheduler front-loads dep-free DMAs)
            nc.vector.tensor_scalar_mul(wo_sb[0:1, 0:1, 0:1],
                                        q_sb[0:1, 0:1, 0:1], 0.0)
            nc.vector.tensor_scalar_mul(masks_sb[0:1, 0:1, 0:1],
                                        q_sb[0:1, 0:1, 0:1], 0.0)
            nc.scalar.dma_start(out=wo_sb[:], in_=wo_e[:])
            nc.gpsimd.dma_start(out=masks_sb[:], in_=masks_d[:])

            pending = [None]     # deferred finalize of the previous group

            def flush_pending():
                if pending[0] is not None:
                    pending[0]()
                    pending[0] = None

            def attn_group(h, blk, bw):
                # i-block of width bw tokens starting at token blk
                nj = (blk + bw) // 128
                nd = blk // 128          # first diagonal-band j-tile
                for jt in range(nj):
                    s_t = ps_s.tile([128, 512], F32, name="s_t", tag="s")
                    nc.tensor.matmul(s_t[:, 0:bw],
                                     k_sb[:, h, bass.ts(jt, 128)],
                                     q_sb[:, h, blk:blk + bw],
                                     start=True, stop=True)
                    o_diag = jt - nd
                    if o_diag >= 0:
                        msk = p2b.tile([128, 512], F32, name="msk", tag="msk")
                        nc.vector.tensor_add(msk[:, 0:bw], s_t[:, 0:bw],
                                             masks_sb[:, o_diag, 0:bw])
                        src = msk
                    else:
                        src = s_t
                    pT = p2.tile([128, 512], BF16, name="pT", tag="pT")
                    nc.scalar.activation(
                        pT[:, 0:bw], src[:, 0:bw],
                        mybir.ActivationFunctionType.Exp, scale=SCALE)
                    if jt == 0:
                        # stage-1 finalize of the previous group lands before
                        # den's WAR on the (bufs=1) den bank
                        flush_pending()
                        den_t = ps_den.tile([1, 512], F32, name="den",
                                            tag="den")
                        av_t = ps_av.tile([128, 512], F32, name="av",
                                          tag="av")
                    nc.tensor.matmul(den_t[:, 0:bw], ones_col[:],
                                     pT[:, 0:bw], start=(jt == 0),
                                     stop=(jt == nj - 1))
                    nc.tensor.matmul(av_t[:, 0:bw],
                                     v_sb[:, jt, bass.ts(h, 128)],
                                     pT[:, 0:bw], start=(jt == 0),
                                     stop=(jt == nj - 1))

                den_f, av_f = den_t, av_t

                def finalize():
                    rden = p2b.tile([1, 512], F32, name="rden", tag="rden")
                    nc.vector.reciprocal_approx_fast(rden[:, 0:bw],
                                                     den_f[:, 0:bw])
                    bc = p2b.tile([128, 512], F32, name="bc", tag="bc")
                    nc.gpsimd.partition_broadcast(bc[:, 0:bw], rden[:, 0:bw])
                    nc.vector.tensor_mul(avT_sb[:, h, blk:blk + bw],
                                         av_f[:, 0:bw], bc[:, 0:bw])

                pending[0] = finalize

            def emit_wo_tile(cb, t):
                base, w = CHUNKS[cb]
                ic, eb = t // 4, t % 4
                im = base // 128 + ic
                wps = ps_wo.tile([128, 512], F32, name="wps", tag="wps")
                for hh in range(HLOC):
                    nc.tensor.matmul(
                        wps[:], avT_sb[:, hh, bass.ts(im, 128)],
                        wo_sb[:, hh, bass.ts(eb, 512)],
                        start=(hh == 0), stop=(hh == HLOC - 1))
                po = p2.tile([128, 512], BF16, name="po", tag="po")
                nc.vector.tensor_scalar_mul(po[:], wps[:], 1.0)
                # po spills ride the gpsimd queue; the sync queue stays empty
                # in phase 2 so the tail out-DMAs can wait there harmlessly
                nc.gpsimd.dma_start(
                    out=part_d[cb][bass.ts(ic, 128), bass.ts(eb, 512)],
                    in_=po[:])

            def emit_rs(cb):
                nc.gpsimd.collective_compute(
                    "ReduceScatter",
                    mybir.AluOpType.add,
                    replica_groups=[[0, 1], [2, 3], [4, 5], [6, 7]],
                    ins=[part_d[cb][:]],
                    outs=[rs_d[cb][:]],
                )

            # i-blocks: three 512-wide, then two 256-wide (finer tail);
            # chunk cb's Wo tiles+RS are interleaved into block cb+1
            BLOCKS = [(0, 512), (512, 512), (1024, 512), (1536, 256),
                      (1792, 256)]
            for bi, (blk, bw) in enumerate(BLOCKS):
                ntiles = CHUNKS[bi - 1][1] // 32 if bi >= 1 else 0
                for h in range(HLOC):
                    attn_group(h, blk, bw)
                    if bi >= 1:       # interleave prev chunk's Wo tiles
                        per = ntiles // HLOC
                        for t in range(per * h, per * (h + 1)):
                            emit_wo_tile(bi - 1, t)
                        if h == HLOC - 1:
                            emit_rs(bi - 1)
            flush_pending()
            for t in range(8):
                emit_wo_tile(4, t)
            emit_rs(4)
            # out DMAs on the (phase-2-idle) sync queue: even when the
            # scheduler hoists out(cb) right behind RS(cb), its RS-done wait
            # at the queue head blocks nothing
            for cb in range(5):
                base, w = CHUNKS[cb]
                o0 = base // GROUPS
                nc.sync.dma_start(out=out_e[o0:o0 + w // GROUPS, :],
                                  in_=rs_d[cb][:])

    nc.compile()
    return nc


def kernel(x, Wq, Wk, Wv, Wo):
    _register_ntff_hook()
    if "nc" not in _cache:
        _cache["nc"] = _build_nc()
    nc = _cache["nc"]

    E = HLOC * HD
    in_maps = []
    for c in CORE_IDS:
        b, g = c // GROUPS, c % GROUPS
        sl = slice(g * E, (g + 1) * E)
        xT = np.ascontiguousarray(x[b].T)                       # [D, S]
        in_maps.append({
            "xs": np.ascontiguousarray(
                xT.reshape(16, 128, 4, 512).transpose(2, 1, 0, 3)).astype(BF),
            "wq": np.ascontiguousarray(
                Wq[sl, :].T.reshape(16, 128, HLOC, 128)
                .transpose(2, 1, 0, 3)).astype(BF),
            "wk": np.ascontiguousarray(
                Wk[sl, :].T.reshape(16, 128, HLOC, 128)
                .transpose(2, 1, 0, 3)).astype(BF),
            "wv": np.ascontiguousarray(
                Wv[sl, :].T.reshape(16, 128, E).transpose(1, 0, 2)).astype(BF),
            "wo": np.ascontiguousarray(
                Wo[:, sl].T.reshape(HLOC, 128, D)
                .transpose(1, 0, 2)).astype(BF),
        })

    trace = bool(os.environ.get("BASS_TRACE"))
    res = run_bass_kernel_spmd(nc, in_maps, CORE_IDS, trace=trace)
    kernel.last_exec_time_ns = res.exec_time_ns
    kernel.last_res = res

    out = np.empty((B, S, D), np.float32)
    chunks = [(0, 512), (512, 512), (1024, 512), (1536, 256), (1792, 256)]
    for c in CORE_IDS:
        b, g = c // GROUPS, c % GROUPS
        r = np.asarray(res.results[c]["out"]).astype(np.float32)  # [1024, D]
        for base, w in chunks:
            half = w // GROUPS
            lo = base + g * half
            out[b, lo:lo + half, :] = r[base // GROUPS:base // GROUPS + half]
    return out


kernel.last_exec_time_ns = None


# revision 32
# speedup vs baseline: 1.2386x; 1.0173x over previous
"""Fused MHA-with-RoPE kernel for one TRN2 chip (8 NeuronCores), v2.

Sharding: core c handles batch b = c//2 and head-group g = c%2 (8 of 16
heads).  All matmul operands are bf16 (fp32 PSUM accumulate), which makes
q/k/v small enough to stay resident in SBUF between phases (no DRAM
spill/reload) and keeps every LDWEIGHTS hidden under its matmul.

  phase 1: QKV projections, sb-outer (x double-buffered per 512-token
           block, w tiles streamed), RoPE fused on the PSUM results,
           q/k stored transposed [hd, S] and v natural [s, hd] in SBUF.
  phase 2: causal attention per (i-block, head), scores computed
           transposed (sT[j,i]) so no PE transposes are needed; softmax
           denominator via ones-matmul accumulated alongside AV; 1/den
           via DVE reciprocal_approx_fast; denominator broadcast via a
           tiny ones-row PE matmul.  Finalize (recip/broadcast/normalize)
           of each head group is deferred past the next group's first
           score tile so the PE never waits on the exp chain.
  phase 3: output projection, interleaved per 512-token chunk inside
           phase 2's i-block loop: chunk cb's 16 Wo tiles are emitted two
           per head during chunk cb+1's attention, then a pair
           ReduceScatter per chunk, so only the last chunk's Wo+RS is on
           the critical-path tail.
Host: shards/transposes/bf16-casts inputs, reassembles RS-interleaved rows.

Self-contained: only numpy/ml_dtypes + concourse + the axon boot shim.
"""

import math
import os
import sys
import types
from contextlib import ExitStack

import ml_dtypes
import numpy as np

import concourse.bass as bass
import concourse.tile as tile
from concourse import bacc, mybir
from concourse.bass_utils import run_bass_kernel_spmd

# ---------------------------------------------------------------- constants
B, S, D = 4, 2048, 2048
H, HD = 16, 128
GROUPS = 2            # head groups (cores per batch)
HLOC = H // GROUPS    # heads per core = 8
N_CORES = 8
CORE_IDS = list(range(N_CORES))
SCALE = 1.0 / math.sqrt(HD)
NEG = -1.0e30
ROPE_BASE = 10000.0

F32 = mybir.dt.float32
BF16 = mybir.dt.bfloat16
BF = ml_dtypes.bfloat16

_cache = {}


def _register_ntff_hook():
    """trn_boot can't register the NTFF profile hook (antenv.axon_hooks is
    missing from this image); recreate it so BASS_TRACE=1 profiling works."""
    if "antenv.axon_hooks" in sys.modules:
        return
    try:
        from trn_agent_boot.trn_boot import _ntff_profile_via_ctypes

        holder = {"h": _ntff_profile_via_ctypes("/opt/axon/libaxon_pjrt.so")}
        mod = types.ModuleType("antenv.axon_hooks")
        mod.get_axon_ntff_profile_hook = lambda: holder["h"]
        mod.set_axon_ntff_profile_hook = lambda h: holder.__setitem__("h", h)
        sys.modules["antenv.axon_hooks"] = mod
    except Exception:
        pass


def _host_tables():
    inv_freq = 1.0 / (ROPE_BASE ** (np.arange(0, HD, 2, dtype=np.float64) / HD))
    pos = np.arange(S, dtype=np.float64)
    freqs = pos[:, None] * inv_freq[None, :]
    emb = np.concatenate([freqs, freqs], axis=-1)        # [S, HD]
    cosT = np.ascontiguousarray(np.cos(emb).T.astype(np.float32))  # [HD, S]
    sinT = np.ascontiguousarray(np.sin(emb).T.astype(np.float32))
    sinF = sinT.copy()
    sinF[: HD // 2] *= -1.0                              # fold rotate_half sign
    return cosT, sinF


def _host_masks():
    # masks[j_local, o, i_local]: 0 if i_local >= o*128 + j_local else NEG
    m = np.empty((128, 4, 512), np.float32)
    jj = np.arange(128)[:, None]
    ii = np.arange(512)[None, :]
    for o in range(4):
        m[:, o, :] = np.where(ii >= o * 128 + jj, 0.0, NEG)
    return m


def _build_nc():
    nc = bacc.Bacc("TRN2", target_bir_lowering=False, debug=False,
                   num_devices=N_CORES)

    # host-pre-tiled bf16 inputs: partition-contiguous DMA layouts
    xs_e = nc.dram_tensor("xs", [4, 128, 16, 512], BF16, kind="ExternalInput")
    wq_e = nc.dram_tensor("wq", [HLOC, 128, 16, 128], BF16,
                          kind="ExternalInput")
    wk_e = nc.dram_tensor("wk", [HLOC, 128, 16, 128], BF16,
                          kind="ExternalInput")
    wv_e = nc.dram_tensor("wv", [128, 16, HLOC * HD], BF16,
                          kind="ExternalInput")
    wo_e = nc.dram_tensor("wo", [128, HLOC, D], BF16, kind="ExternalInput")
    # bf16, written directly by the ReduceScatters; host converts to f32.
    # rows: per 512-chunk cb 0-2 -> 256 rows; 256-chunks 3a/3b -> 128 rows
    out_e = nc.dram_tensor("out", [S // GROUPS, D], BF16,
                           kind="ExternalOutput")

    cosT_d = nc.inline_tensor(_host_tables()[0], name="cosT")
    sinF_d = nc.inline_tensor(_host_tables()[1], name="sinF")
    masks_d = nc.inline_tensor(_host_masks(), name="masks")

    with tile.TileContext(nc) as tc, ExitStack() as ctx:
        dram = ctx.enter_context(tc.tile_pool(name="dram", bufs=1, space="DRAM"))
        # chunks 0-2: 512 tokens; 3a/3b: 256 tokens (finer tail)
        CHUNKS = [(0, 512), (512, 512), (1024, 512), (1536, 256), (1792, 256)]
        part_d = [dram.tile([w, D], BF16, name=f"part_d{i}")
                  for i, (_, w) in enumerate(CHUNKS)]
        rs_d = [dram.tile([w // GROUPS, D], BF16, name=f"rs_d{i}")
                for i, (_, w) in enumerate(CHUNKS)]

        # persistent across phases: ones vectors + q/k/v in SBUF
        consts = ctx.enter_context(tc.tile_pool(name="consts", bufs=1))
        ones_col = consts.tile([128, 1], BF16)
        nc.vector.memset(ones_col[:], 1.0)

        qkv = ctx.enter_context(tc.tile_pool(name="qkv", bufs=1))
        q_sb = qkv.tile([128, HLOC, S], BF16)     # [hd, h, s]
        k_sb = qkv.tile([128, HLOC, S], BF16)     # [hd, h, s]
        v_sb = qkv.tile([128, 16, HLOC * HD], BF16)  # [s_sub, st, h*hd]

        # ---------------- phase 1: projections ----------------
        with tc.tile_pool(name="tabs", bufs=1) as tabs, \
             tc.tile_pool(name="xT", bufs=2) as xT_pool, \
             tc.tile_pool(name="wqk", bufs=3) as wqk_pool, \
             tc.tile_pool(name="wv", bufs=1) as wv_pool, \
             tc.tile_pool(name="rope", bufs=2) as rope, \
             tc.tile_pool(name="ps1", bufs=4, space="PSUM") as ps1:
            cos_sb = tabs.tile([HD, S], F32)
            sinF_sb = tabs.tile([HD, S], F32)
            nc.gpsimd.dma_start(out=cos_sb[:], in_=cosT_d[:])
            nc.gpsimd.dma_start(out=sinF_sb[:], in_=sinF_d[:])
            wv_sb = wv_pool.tile([128, 16, HLOC * HD], BF16)

            HF = HD // 2
            for sb in range(4):
                xt = xT_pool.tile([128, 16, 512], BF16, name=f"xt{sb}",
                                  tag="xt")
                if sb > 0:
                    # gate later x loads behind first phase-1 output so the
                    # scheduler can't front-load them against the critical
                    # first-tile DMAs
                    nc.vector.tensor_scalar_mul(xt[0:1, 0:1, 0:1],
                                                q_sb[0:1, 0:1, 0:1], 0.0)
                    for qd in range(4):
                        nc.sync.dma_start(out=xt[:, 4 * qd:4 * qd + 4, :],
                                          in_=xs_e[sb, :, 4 * qd:4 * qd + 4, :])
                c_sl = cos_sb[:, bass.ts(sb, 512)]
                s_sl = sinF_sb[:, bass.ts(sb, 512)]
                # q/k projections + RoPE, written transposed [hd, h, s]
                for w_e, o_sb, pname in ((wq_e, q_sb, "q"), (wk_e, k_sb, "k")):
                    for m in range(HLOC):
                        w_t = wqk_pool.tile([128, 16, 128], BF16,
                                            name=f"w{pname}{m}", tag="w")
                        if sb == 0 and pname == "q" and m == 0:
                            # per-dt loads: slice-granular deps let matmul dt
                            # start as soon as its own 160KB arrived
                            for dt_ in range(16):
                                nc.sync.dma_start(
                                    out=xt[:, dt_:dt_ + 1, :],
                                    in_=xs_e[0, :, dt_:dt_ + 1, :])
                                nc.sync.dma_start(
                                    out=w_t[:, dt_:dt_ + 1, :],
                                    in_=w_e[0, :, dt_:dt_ + 1, :])
                        else:
                            nc.sync.dma_start(out=w_t[:], in_=w_e[m])
                        ps = ps1.tile([128, 512], F32, name="ps_qk",
                                      tag="ps1")
                        for dt_ in range(16):
                            nc.tensor.matmul(
                                ps[:], w_t[:, dt_, :], xt[:, dt_, :],
                                start=(dt_ == 0), stop=(dt_ == 15))
                        sw = rope.tile([128, 512], F32, name="sw", tag="sw")
                        nc.scalar.copy(sw[0:HF, :], ps[HF:HD, :])
                        nc.scalar.copy(sw[HF:HD, :], ps[0:HF, :])
                        m1 = rope.tile([128, 512], F32, name="m1", tag="m1")
                        nc.vector.tensor_mul(m1[:], ps[:], c_sl)
                        m2 = rope.tile([128, 512], F32, name="m2", tag="m2")
                        nc.vector.tensor_mul(m2[:], sw[:], s_sl)
                        nc.vector.tensor_add(
                            o_sb[:, m, bass.ts(sb, 512)], m1[:], m2[:])
                        if sb == 0 and pname == "q" and m == 0:
                            # gate the wv bulk load off the first ~10us too
                            nc.vector.tensor_scalar_mul(
                                wv_sb[0:1, 0:1, 0:1], q_sb[0:1, 0:1, 0:1],
                                0.0)
                            nc.scalar.dma_start(out=wv_sb[:], in_=wv_e[:])
                # v projection: x tile stationary, wv moving (512-row MMs)
                for ss in range(4):
                    st = sb * 4 + ss
                    for eh in range(2):
                        ps = ps1.tile([128, 512], F32, name="ps_v", tag="ps1")
                        for dt_ in range(16):
                            nc.tensor.matmul(
                                ps[:], xt[:, dt_, bass.ts(ss, 128)],
                                wv_sb[:, dt_, bass.ts(eh, 512)],
                                start=(dt_ == 0), stop=(dt_ == 15))
                        nc.scalar.copy(v_sb[:, st, bass.ts(eh, 512)], ps[:])

        # ---------------- phase 2+3: attention + Wo + RS ----------------
        with tc.tile_pool(name="wo", bufs=1) as wo_pool, \
             tc.tile_pool(name="avT", bufs=1) as avT_pool, \
             tc.tile_pool(name="msks", bufs=1) as msks_pool, \
             tc.tile_pool(name="p2", bufs=3) as p2, \
             tc.tile_pool(name="p2b", bufs=2) as p2b, \
             tc.tile_pool(name="ps_s", bufs=3, space="PSUM") as ps_s, \
             tc.tile_pool(name="ps_av", bufs=2, space="PSUM") as ps_av, \
             tc.tile_pool(name="ps_den", bufs=1, space="PSUM") as ps_den, \
             tc.tile_pool(name="ps_wo", bufs=2, space="PSUM") as ps_wo:
            wo_sb = wo_pool.tile([128, HLOC, D], BF16)
            avT_sb = avT_pool.tile([128, HLOC, S], BF16)
            masks_sb = msks_pool.tile([128, 4, 512], F32)
            # dummy writes depending on phase-1 data gate these bulk loads
            # off the critical first ~30us of input DMA bandwidth (the
            # scheduler front-loads dep-free DMAs)
            nc.vector.tensor_scalar_mul(wo_sb[0:1, 0:1, 0:1],
                                        q_sb[0:1, 0:1, 0:1], 0.0)
            nc.vector.tensor_scalar_mul(masks_sb[0:1, 0:1, 0:1],
                                        q_sb[0:1, 0:1, 0:1], 0.0)
            nc.scalar.dma_start(out=wo_sb[:], in_=wo_e[:])
            nc.gpsimd.dma_start(out=masks_sb[:], in_=masks_d[:])

            pending = [None]     # deferred finalize of the previous group

            def flush_pending():
                if pending[0] is not None:
                    pending[0]()
                    pending[0] = None

            def attn_group(h, blk, bw):
                # i-block of width bw tokens starting at token blk.
                # bw==256 pairs two pT tiles side by side in one [128,512]
                # tile so den runs one 512-row matmul per pair instead of
                # two 256-row ones (fixed per-matmul overhead dominates
                # small matmuls); the two partial dens are summed in
                # finalize.
                nj = (blk + bw) // 128
                nd = blk // 128          # first diagonal-band j-tile
                pair = bw == 256
                pT = None
                for jt in range(nj):
                    s_t = ps_s.tile([128, 512], F32, name="s_t", tag="s")
                    nc.tensor.matmul(s_t[:, 0:bw],
                                     k_sb[:, h, bass.ts(jt, 128)],
                                     q_sb[:, h, blk:blk + bw],
                                     start=True, stop=True)
                    o_diag = jt - nd
                    if o_diag >= 0:
                        msk = p2b.tile([128, 512], F32, name="msk", tag="msk")
                        nc.vector.tensor_add(msk[:, 0:bw], s_t[:, 0:bw],
                                             masks_sb[:, o_diag, 0:bw])
                        src = msk
                    else:
                        src = s_t
                    if not pair or jt % 2 == 0:
                        pT = p2.tile([128, 512], BF16, name="pT", tag="pT")
                    half = bass.ts(jt % 2, bw) if pair else slice(0, bw)
                    nc.scalar.activation(
                        pT[:, half], src[:, 0:bw],
                        mybir.ActivationFunctionType.Exp, scale=SCALE)
                    if jt == 0:
                        # stage-1 finalize of the previous group lands before
                        # den's WAR on the (bufs=1) den bank
                        flush_pending()
                        den_t = ps_den.tile([1, 512], F32, name="den",
                                            tag="den")
                        av_t = ps_av.tile([128, 512], F32, name="av",
                                          tag="av")
                    if not pair:
                        nc.tensor.matmul(den_t[:, 0:bw], ones_col[:],
                                         pT[:, 0:bw], start=(jt == 0),
                                         stop=(jt == nj - 1))
                    elif jt % 2 == 1:
                        nc.tensor.matmul(den_t[:, 0:2 * bw], ones_col[:],
                                         pT[:, 0:2 * bw], start=(jt == 1),
                                         stop=(jt == nj - 1))
                    nc.tensor.matmul(av_t[:, 0:bw],
                                     v_sb[:, jt, bass.ts(h, 128)],
                                     pT[:, half], start=(jt == 0),
                                     stop=(jt == nj - 1))

                den_f, av_f = den_t, av_t

                def finalize():
                    if pair:
                        # den halves live at columns [0:bw] (even jt) and
                        # [bw:2bw] (odd jt): copy out of PSUM, sum halves
                        den_sb = p2b.tile([1, 512], F32, name="den_sb",
                                          tag="den_sb")
                        nc.scalar.copy(den_sb[:, 0:2 * bw],
                                       den_f[:, 0:2 * bw])
                        dsum = p2b.tile([1, 512], F32, name="dsum",
                                        tag="dsum")
                        nc.vector.tensor_add(dsum[:, 0:bw],
                                             den_sb[:, 0:bw],
                                             den_sb[:, bw:2 * bw])
                        rsrc = dsum
                    else:
                        rsrc = den_f
                    rden = p2b.tile([1, 512], F32, name="rden", tag="rden")
                    nc.vector.reciprocal_approx_fast(rden[:, 0:bw],
                                                     rsrc[:, 0:bw])
                    bc = p2b.tile([128, 512], F32, name="bc", tag="bc")
                    nc.gpsimd.partition_broadcast(bc[:, 0:bw], rden[:, 0:bw])
                    nc.vector.tensor_mul(avT_sb[:, h, blk:blk + bw],
                                         av_f[:, 0:bw], bc[:, 0:bw])

                pending[0] = finalize

            def emit_wo_tile(cb, t):
                base, w = CHUNKS[cb]
                ic, eb = t // 4, t % 4
                im = base // 128 + ic
                wps = ps_wo.tile([128, 512], F32, name="wps", tag="wps")
                for hh in range(HLOC):
                    nc.tensor.matmul(
                        wps[:], avT_sb[:, hh, bass.ts(im, 128)],
                        wo_sb[:, hh, bass.ts(eb, 512)],
                        start=(hh == 0), stop=(hh == HLOC - 1))
                po = p2.tile([128, 512], BF16, name="po", tag="po")
                nc.vector.tensor_scalar_mul(po[:], wps[:], 1.0)
                # po spills ride the gpsimd queue; the sync queue stays empty
                # in phase 2 so the tail out-DMAs can wait there harmlessly
                nc.gpsimd.dma_start(
                    out=part_d[cb][bass.ts(ic, 128), bass.ts(eb, 512)],
                    in_=po[:])

            def emit_rs(cb):
                nc.gpsimd.collective_compute(
                    "ReduceScatter",
                    mybir.AluOpType.add,
                    replica_groups=[[0, 1], [2, 3], [4, 5], [6, 7]],
                    ins=[part_d[cb][:]],
                    outs=[rs_d[cb][:]],
                )

            # i-blocks: three 512-wide, then two 256-wide (finer tail);
            # chunk cb's Wo tiles+RS are interleaved into block cb+1
            BLOCKS = [(0, 512), (512, 512), (1024, 512), (1536, 256),
                      (1792, 256)]
            for bi, (blk, bw) in enumerate(BLOCKS):
                ntiles = CHUNKS[bi - 1][1] // 32 if bi >= 1 else 0
                for h in range(HLOC):
                    attn_group(h, blk, bw)
                    if bi >= 1:       # interleave prev chunk's Wo tiles
                        per = ntiles // HLOC
                        for t in range(per * h, per * (h + 1)):
                            emit_wo_tile(bi - 1, t)
                        if h == HLOC - 1:
                            emit_rs(bi - 1)
            flush_pending()
            for t in range(8):
                emit_wo_tile(4, t)
            emit_rs(4)
            # out DMAs on the (phase-2-idle) sync queue: even when the
            # scheduler hoists out(cb) right behind RS(cb), its RS-done wait
            # at the queue head blocks nothing
            for cb in range(5):
                base, w = CHUNKS[cb]
                o0 = base // GROUPS
                nc.sync.dma_start(out=out_e[o0:o0 + w // GROUPS, :],
                                  in_=rs_d[cb][:])

    nc.compile()
    return nc


def kernel(x, Wq, Wk, Wv, Wo):
    _register_ntff_hook()
    if "nc" not in _cache:
        _cache["nc"] = _build_nc()
    nc = _cache["nc"]

    E = HLOC * HD
    in_maps = []
    for c in CORE_IDS:
        b, g = c // GROUPS, c % GROUPS
        sl = slice(g * E, (g + 1) * E)
        xT = np.ascontiguousarray(x[b].T)                       # [D, S]
        in_maps.append({
            "xs": np.ascontiguousarray(
                xT.reshape(16, 128, 4, 512).transpose(2, 1, 0, 3)).astype(BF),
            "wq": np.ascontiguousarray(
                Wq[sl, :].T.reshape(16, 128, HLOC, 128)
                .transpose(2, 1, 0, 3)).astype(BF),
            "wk": np.ascontiguousarray(
                Wk[sl, :].T.reshape(16, 128, HLOC, 128)
                .transpose(2, 1, 0, 3)).astype(BF),
            "wv": np.ascontiguousarray(
                Wv[sl, :].T.reshape(16, 128, E).transpose(1, 0, 2)).astype(BF),
            "wo": np.ascontiguousarray(
                Wo[:, sl].T.reshape(HLOC, 128, D)
                .transpose(1, 0, 2)).astype(BF),
        })

    trace = bool(os.environ.get("BASS_TRACE"))
    res = run_bass_kernel_spmd(nc, in_maps, CORE_IDS, trace=trace)
    kernel.last_exec_time_ns = res.exec_time_ns
    kernel.last_res = res

    out = np.empty((B, S, D), np.float32)
    chunks = [(0, 512), (512, 512), (1024, 512), (1536, 256), (1792, 256)]
    for c in CORE_IDS:
        b, g = c // GROUPS, c % GROUPS
        r = np.asarray(res.results[c]["out"]).astype(np.float32)  # [1024, D]
        for base, w in chunks:
            half = w // GROUPS
            lo = base + g * half
            out[b, lo:lo + half, :] = r[base // GROUPS:base // GROUPS + half]
    return out


kernel.last_exec_time_ns = None


# revision 34
# speedup vs baseline: 1.3339x; 1.0770x over previous
"""Fused MHA-with-RoPE kernel for one TRN2 chip (8 NeuronCores), v2.

Sharding: core c handles batch b = c//2 and head-group g = c%2 (8 of 16
heads).  All matmul operands are bf16 (fp32 PSUM accumulate), which makes
q/k/v small enough to stay resident in SBUF between phases (no DRAM
spill/reload) and keeps every LDWEIGHTS hidden under its matmul.

  phase 1: QKV projections, sb-outer (x double-buffered per 512-token
           block, w tiles streamed), RoPE fused on the PSUM results,
           q/k stored transposed [hd, S] and v natural [s, hd] in SBUF.
  phase 2: causal attention per (i-block, head), scores computed
           transposed (sT[j,i]) so no PE transposes are needed; softmax
           denominator via ones-matmul accumulated alongside AV; 1/den
           via DVE reciprocal_approx_fast; denominator broadcast via a
           tiny ones-row PE matmul.  Finalize (recip/broadcast/normalize)
           of each head group is deferred past the next group's first
           score tile so the PE never waits on the exp chain.
  phase 3: output projection, interleaved per 512-token chunk inside
           phase 2's i-block loop: chunk cb's 16 Wo tiles are emitted two
           per head during chunk cb+1's attention, then a pair
           ReduceScatter per chunk, so only the last chunk's Wo+RS is on
           the critical-path tail.
Host: shards/transposes/bf16-casts inputs, reassembles RS-interleaved rows.

Self-contained: only numpy/ml_dtypes + concourse + the axon boot shim.
"""

import math
import os
import sys
import types
from contextlib import ExitStack

import ml_dtypes
import numpy as np

import concourse.bass as bass
import concourse.tile as tile
from concourse import bacc, mybir
from concourse.bass_utils import run_bass_kernel_spmd

# ---------------------------------------------------------------- constants
B, S, D = 4, 2048, 2048
H, HD = 16, 128
GROUPS = 2            # head groups (cores per batch)
HLOC = H // GROUPS    # heads per core = 8
N_CORES = 8
CORE_IDS = list(range(N_CORES))
SCALE = 1.0 / math.sqrt(HD)
NEG = -1.0e30
ROPE_BASE = 10000.0

F32 = mybir.dt.float32
BF16 = mybir.dt.bfloat16
BF = ml_dtypes.bfloat16

_cache = {}


def _register_ntff_hook():
    """trn_boot can't register the NTFF profile hook (antenv.axon_hooks is
    missing from this image); recreate it so BASS_TRACE=1 profiling works."""
    if "antenv.axon_hooks" in sys.modules:
        return
    try:
        from trn_agent_boot.trn_boot import _ntff_profile_via_ctypes

        holder = {"h": _ntff_profile_via_ctypes("/opt/axon/libaxon_pjrt.so")}
        mod = types.ModuleType("antenv.axon_hooks")
        mod.get_axon_ntff_profile_hook = lambda: holder["h"]
        mod.set_axon_ntff_profile_hook = lambda h: holder.__setitem__("h", h)
        sys.modules["antenv.axon_hooks"] = mod
    except Exception:
        pass


def _host_tables():
    inv_freq = 1.0 / (ROPE_BASE ** (np.arange(0, HD, 2, dtype=np.float64) / HD))
    pos = np.arange(S, dtype=np.float64)
    freqs = pos[:, None] * inv_freq[None, :]
    emb = np.concatenate([freqs, freqs], axis=-1)        # [S, HD]
    cosT = np.ascontiguousarray(np.cos(emb).T.astype(np.float32))  # [HD, S]
    sinT = np.ascontiguousarray(np.sin(emb).T.astype(np.float32))
    sinF = sinT.copy()
    sinF[: HD // 2] *= -1.0                              # fold rotate_half sign
    return cosT, sinF


def _host_masks():
    # masks[j_local, o, i_local]: 0 if i_local >= o*128 + j_local else NEG
    m = np.empty((128, 4, 512), np.float32)
    jj = np.arange(128)[:, None]
    ii = np.arange(512)[None, :]
    for o in range(4):
        m[:, o, :] = np.where(ii >= o * 128 + jj, 0.0, NEG)
    return m


def _build_nc():
    nc = bacc.Bacc("TRN2", target_bir_lowering=False, debug=False,
                   num_devices=N_CORES)

    # host-pre-tiled bf16 inputs: partition-contiguous DMA layouts
    xs_e = nc.dram_tensor("xs", [4, 128, 16, 512], BF16, kind="ExternalInput")
    wq_e = nc.dram_tensor("wq", [HLOC, 128, 16, 128], BF16,
                          kind="ExternalInput")
    wk_e = nc.dram_tensor("wk", [HLOC, 128, 16, 128], BF16,
                          kind="ExternalInput")
    wv_e = nc.dram_tensor("wv", [128, 16, HLOC * HD], BF16,
                          kind="ExternalInput")
    wo_e = nc.dram_tensor("wo", [128, HLOC, D], BF16, kind="ExternalInput")
    # bf16, written directly by the ReduceScatters; host converts to f32.
    # rows: per 512-chunk cb 0-2 -> 256 rows; 256-chunks 3a/3b -> 128 rows
    out_e = nc.dram_tensor("out", [S // GROUPS, D], BF16,
                           kind="ExternalOutput")

    cosT_d = nc.inline_tensor(_host_tables()[0], name="cosT")
    sinF_d = nc.inline_tensor(_host_tables()[1], name="sinF")
    masks_d = nc.inline_tensor(_host_masks(), name="masks")

    with tile.TileContext(nc) as tc, ExitStack() as ctx:
        dram = ctx.enter_context(tc.tile_pool(name="dram", bufs=1, space="DRAM"))
        # chunks 0-2: 512 tokens; 3a/3b: 256 tokens (finer tail)
        CHUNKS = [(0, 512), (512, 512), (1024, 512), (1536, 256), (1792, 256)]
        part_d = [dram.tile([w, D], BF16, name=f"part_d{i}")
                  for i, (_, w) in enumerate(CHUNKS)]
        rs_d = [dram.tile([w // GROUPS, D], BF16, name=f"rs_d{i}")
                for i, (_, w) in enumerate(CHUNKS)]

        # persistent across phases: ones + q/k/v in SBUF.  ones is a full
        # [128,128] block: the den matmul then yields den replicated across
        # all partitions (a normal-speed matmul, and no partition_broadcast
        # needed to normalize)
        consts = ctx.enter_context(tc.tile_pool(name="consts", bufs=1))
        ones_f = consts.tile([128, 128], BF16)
        nc.vector.memset(ones_f[:], 1.0)

        qkv = ctx.enter_context(tc.tile_pool(name="qkv", bufs=1))
        q_sb = qkv.tile([128, HLOC, S], BF16)     # [hd, h, s]
        k_sb = qkv.tile([128, HLOC, S], BF16)     # [hd, h, s]
        v_sb = qkv.tile([128, 16, HLOC * HD], BF16)  # [s_sub, st, h*hd]

        # ---------------- phase 1: projections ----------------
        with tc.tile_pool(name="tabs", bufs=1) as tabs, \
             tc.tile_pool(name="xT", bufs=2) as xT_pool, \
             tc.tile_pool(name="wqk", bufs=3) as wqk_pool, \
             tc.tile_pool(name="wv", bufs=1) as wv_pool, \
             tc.tile_pool(name="rope", bufs=2) as rope, \
             tc.tile_pool(name="ps1", bufs=4, space="PSUM") as ps1:
            cos_sb = tabs.tile([HD, S], F32)
            sinF_sb = tabs.tile([HD, S], F32)
            nc.gpsimd.dma_start(out=cos_sb[:], in_=cosT_d[:])
            nc.gpsimd.dma_start(out=sinF_sb[:], in_=sinF_d[:])
            wv_sb = wv_pool.tile([128, 16, HLOC * HD], BF16)

            HF = HD // 2
            for sb in range(4):
                xt = xT_pool.tile([128, 16, 512], BF16, name=f"xt{sb}",
                                  tag="xt")
                if sb > 0:
                    # gate later x loads behind first phase-1 output so the
                    # scheduler can't front-load them against the critical
                    # first-tile DMAs
                    nc.vector.tensor_scalar_mul(xt[0:1, 0:1, 0:1],
                                                q_sb[0:1, 0:1, 0:1], 0.0)
                    for qd in range(4):
                        nc.sync.dma_start(out=xt[:, 4 * qd:4 * qd + 4, :],
                                          in_=xs_e[sb, :, 4 * qd:4 * qd + 4, :])
                c_sl = cos_sb[:, bass.ts(sb, 512)]
                s_sl = sinF_sb[:, bass.ts(sb, 512)]
                # q/k projections + RoPE, written transposed [hd, h, s]
                for w_e, o_sb, pname in ((wq_e, q_sb, "q"), (wk_e, k_sb, "k")):
                    for m in range(HLOC):
                        w_t = wqk_pool.tile([128, 16, 128], BF16,
                                            name=f"w{pname}{m}", tag="w")
                        if sb == 0 and pname == "q" and m == 0:
                            # per-dt loads: slice-granular deps let matmul dt
                            # start as soon as its own 160KB arrived
                            for dt_ in range(16):
                                nc.sync.dma_start(
                                    out=xt[:, dt_:dt_ + 1, :],
                                    in_=xs_e[0, :, dt_:dt_ + 1, :])
                                nc.sync.dma_start(
                                    out=w_t[:, dt_:dt_ + 1, :],
                                    in_=w_e[0, :, dt_:dt_ + 1, :])
                        else:
                            nc.sync.dma_start(out=w_t[:], in_=w_e[m])
                        ps = ps1.tile([128, 512], F32, name="ps_qk",
                                      tag="ps1")
                        for dt_ in range(16):
                            nc.tensor.matmul(
                                ps[:], w_t[:, dt_, :], xt[:, dt_, :],
                                start=(dt_ == 0), stop=(dt_ == 15))
                        sw = rope.tile([128, 512], F32, name="sw", tag="sw")
                        nc.scalar.copy(sw[0:HF, :], ps[HF:HD, :])
                        nc.scalar.copy(sw[HF:HD, :], ps[0:HF, :])
                        m1 = rope.tile([128, 512], F32, name="m1", tag="m1")
                        nc.vector.tensor_mul(m1[:], ps[:], c_sl)
                        m2 = rope.tile([128, 512], F32, name="m2", tag="m2")
                        nc.vector.tensor_mul(m2[:], sw[:], s_sl)
                        nc.vector.tensor_add(
                            o_sb[:, m, bass.ts(sb, 512)], m1[:], m2[:])
                        if sb == 0 and pname == "q" and m == 0:
                            # gate the wv bulk load off the first ~10us too
                            nc.vector.tensor_scalar_mul(
                                wv_sb[0:1, 0:1, 0:1], q_sb[0:1, 0:1, 0:1],
                                0.0)
                            nc.scalar.dma_start(out=wv_sb[:], in_=wv_e[:])
                # v projection: x tile stationary, wv moving (512-row MMs)
                for ss in range(4):
                    st = sb * 4 + ss
                    for eh in range(2):
                        ps = ps1.tile([128, 512], F32, name="ps_v", tag="ps1")
                        for dt_ in range(16):
                            nc.tensor.matmul(
                                ps[:], xt[:, dt_, bass.ts(ss, 128)],
                                wv_sb[:, dt_, bass.ts(eh, 512)],
                                start=(dt_ == 0), stop=(dt_ == 15))
                        nc.scalar.copy(v_sb[:, st, bass.ts(eh, 512)], ps[:])

        # ---------------- phase 2+3: attention + Wo + RS ----------------
        with tc.tile_pool(name="wo", bufs=1) as wo_pool, \
             tc.tile_pool(name="avT", bufs=1) as avT_pool, \
             tc.tile_pool(name="msks", bufs=1) as msks_pool, \
             tc.tile_pool(name="p2", bufs=3) as p2, \
             tc.tile_pool(name="p2b", bufs=2) as p2b, \
             tc.tile_pool(name="ps_s", bufs=3, space="PSUM") as ps_s, \
             tc.tile_pool(name="ps_av", bufs=2, space="PSUM") as ps_av, \
             tc.tile_pool(name="ps_den", bufs=1, space="PSUM") as ps_den, \
             tc.tile_pool(name="ps_wo", bufs=2, space="PSUM") as ps_wo:
            wo_sb = wo_pool.tile([128, HLOC, D], BF16)
            avT_sb = avT_pool.tile([128, HLOC, S], BF16)
            masks_sb = msks_pool.tile([128, 4, 512], F32)
            # dummy writes depending on phase-1 data gate these bulk loads
            # off the critical first ~30us of input DMA bandwidth (the
            # scheduler front-loads dep-free DMAs)
            nc.vector.tensor_scalar_mul(wo_sb[0:1, 0:1, 0:1],
                                        q_sb[0:1, 0:1, 0:1], 0.0)
            nc.vector.tensor_scalar_mul(masks_sb[0:1, 0:1, 0:1],
                                        q_sb[0:1, 0:1, 0:1], 0.0)
            nc.scalar.dma_start(out=wo_sb[:], in_=wo_e[:])
            nc.gpsimd.dma_start(out=masks_sb[:], in_=masks_d[:])

            pending = [None]     # deferred finalize of the previous group

            def flush_pending():
                if pending[0] is not None:
                    pending[0]()
                    pending[0] = None

            def attn_group(h, blk, bw):
                # i-block of width bw tokens starting at token blk
                nj = (blk + bw) // 128
                nd = blk // 128          # first diagonal-band j-tile
                for jt in range(nj):
                    s_t = ps_s.tile([128, 512], F32, name="s_t", tag="s")
                    nc.tensor.matmul(s_t[:, 0:bw],
                                     k_sb[:, h, bass.ts(jt, 128)],
                                     q_sb[:, h, blk:blk + bw],
                                     start=True, stop=True)
                    o_diag = jt - nd
                    if o_diag >= 0:
                        msk = p2b.tile([128, 512], F32, name="msk", tag="msk")
                        nc.vector.tensor_add(msk[:, 0:bw], s_t[:, 0:bw],
                                             masks_sb[:, o_diag, 0:bw])
                        src = msk
                    else:
                        src = s_t
                    pT = p2.tile([128, 512], BF16, name="pT", tag="pT")
                    nc.scalar.activation(
                        pT[:, 0:bw], src[:, 0:bw],
                        mybir.ActivationFunctionType.Exp, scale=SCALE)
                    if jt == 0:
                        # stage-1 finalize of the previous group lands before
                        # den's WAR on the (bufs=1) den bank
                        flush_pending()
                        den_t = ps_den.tile([128, 512], F32, name="den",
                                            tag="den")
                        av_t = ps_av.tile([128, 512], F32, name="av",
                                          tag="av")
                    nc.tensor.matmul(den_t[:, 0:bw], ones_f[:],
                                     pT[:, 0:bw], start=(jt == 0),
                                     stop=(jt == nj - 1))
                    nc.tensor.matmul(av_t[:, 0:bw],
                                     v_sb[:, jt, bass.ts(h, 128)],
                                     pT[:, 0:bw], start=(jt == 0),
                                     stop=(jt == nj - 1))

                den_f, av_f = den_t, av_t

                def finalize():
                    # den is replicated across partitions: reciprocal
                    # directly, no broadcast step
                    rden = p2b.tile([128, 512], F32, name="rden", tag="rden")
                    nc.vector.reciprocal_approx_fast(rden[:, 0:bw],
                                                     den_f[:, 0:bw])
                    nc.vector.tensor_mul(avT_sb[:, h, blk:blk + bw],
                                         av_f[:, 0:bw], rden[:, 0:bw])

                pending[0] = finalize

            def emit_wo_tile(cb, t):
                base, w = CHUNKS[cb]
                ic, eb = t // 4, t % 4
                im = base // 128 + ic
                wps = ps_wo.tile([128, 512], F32, name="wps", tag="wps")
                for hh in range(HLOC):
                    nc.tensor.matmul(
                        wps[:], avT_sb[:, hh, bass.ts(im, 128)],
                        wo_sb[:, hh, bass.ts(eb, 512)],
                        start=(hh == 0), stop=(hh == HLOC - 1))
                po = p2.tile([128, 512], BF16, name="po", tag="po")
                nc.vector.tensor_scalar_mul(po[:], wps[:], 1.0)
                # po spills ride the gpsimd queue; the sync queue stays empty
                # in phase 2 so the tail out-DMAs can wait there harmlessly
                nc.gpsimd.dma_start(
                    out=part_d[cb][bass.ts(ic, 128), bass.ts(eb, 512)],
                    in_=po[:])

            def emit_rs(cb):
                nc.gpsimd.collective_compute(
                    "ReduceScatter",
                    mybir.AluOpType.add,
                    replica_groups=[[0, 1], [2, 3], [4, 5], [6, 7]],
                    ins=[part_d[cb][:]],
                    outs=[rs_d[cb][:]],
                )

            # i-blocks: three 512-wide, then two 256-wide (finer tail);
            # chunk cb's Wo tiles+RS are interleaved into block cb+1
            BLOCKS = [(0, 512), (512, 512), (1024, 512), (1536, 256),
                      (1792, 256)]
            for bi, (blk, bw) in enumerate(BLOCKS):
                ntiles = CHUNKS[bi - 1][1] // 32 if bi >= 1 else 0
                for h in range(HLOC):
                    attn_group(h, blk, bw)
                    if bi >= 1:       # interleave prev chunk's Wo tiles
                        per = ntiles // HLOC
                        for t in range(per * h, per * (h + 1)):
                            emit_wo_tile(bi - 1, t)
                        if h == HLOC - 1:
                            emit_rs(bi - 1)
            flush_pending()
            for t in range(8):
                emit_wo_tile(4, t)
            emit_rs(4)
            # out DMAs on the (phase-2-idle) sync queue: even when the
            # scheduler hoists out(cb) right behind RS(cb), its RS-done wait
            # at the queue head blocks nothing
            for cb in range(5):
                base, w = CHUNKS[cb]
                o0 = base // GROUPS
                nc.sync.dma_start(out=out_e[o0:o0 + w // GROUPS, :],
                                  in_=rs_d[cb][:])

    nc.compile()
    return nc


def kernel(x, Wq, Wk, Wv, Wo):
    _register_ntff_hook()
    if "nc" not in _cache:
        _cache["nc"] = _build_nc()
    nc = _cache["nc"]

    E = HLOC * HD
    in_maps = []
    for c in CORE_IDS:
        b, g = c // GROUPS, c % GROUPS
        sl = slice(g * E, (g + 1) * E)
        xT = np.ascontiguousarray(x[b].T)                       # [D, S]
        in_maps.append({
            "xs": np.ascontiguousarray(
                xT.reshape(16, 128, 4, 512).transpose(2, 1, 0, 3)).astype(BF),
            "wq": np.ascontiguousarray(
                Wq[sl, :].T.reshape(16, 128, HLOC, 128)
                .transpose(2, 1, 0, 3)).astype(BF),
            "wk": np.ascontiguousarray(
                Wk[sl, :].T.reshape(16, 128, HLOC, 128)
                .transpose(2, 1, 0, 3)).astype(BF),
            "wv": np.ascontiguousarray(
                Wv[sl, :].T.reshape(16, 128, E).transpose(1, 0, 2)).astype(BF),
            "wo": np.ascontiguousarray(
                Wo[:, sl].T.reshape(HLOC, 128, D)
                .transpose(1, 0, 2)).astype(BF),
        })

    trace = bool(os.environ.get("BASS_TRACE"))
    res = run_bass_kernel_spmd(nc, in_maps, CORE_IDS, trace=trace)
    kernel.last_exec_time_ns = res.exec_time_ns
    kernel.last_res = res

    out = np.empty((B, S, D), np.float32)
    chunks = [(0, 512), (512, 512), (1024, 512), (1536, 256), (1792, 256)]
    for c in CORE_IDS:
        b, g = c // GROUPS, c % GROUPS
        r = np.asarray(res.results[c]["out"]).astype(np.float32)  # [1024, D]
        for base, w in chunks:
            half = w // GROUPS
            lo = base + g * half
            out[b, lo:lo + half, :] = r[base // GROUPS:base // GROUPS + half]
    return out


kernel.last_exec_time_ns = None
